# revision 27
# baseline (speedup 1.0000x reference)
"""AttentionBlock (GroupNorm -> qkv conv1x1 -> 4-head attention -> proj -> residual)
as a distributed Bass/Tile kernel on 8 TRN2 NeuronCores.

Sharding: core j handles batch b = j//2 and query-half h = j%2. The host
permutes x's spatial columns per core so queries are always cols 0:2048
(attention is permutation-invariant over keys). K/V are computed full-length
per core, so output slices are disjoint and no collectives are needed.

PV and the qkv convs run in fp8 DoubleRow (contraction = 2x128 per pass at
0.5 cyc/col): V is stored e4m3 with the per-head k-pair layout [128, 2, 80]
(j-stride %16==0 per the s3_lw dual-fp8 ISA rule), probabilities are e5m2,
conv weights/inputs e4m3 (x8 derived on device from the bf16 x). Softmax exp
is split between ScalarE (real Exp, fp8e5 out) and VectorE (Schraudolph
bit-trick: uint8 round of a*s+b ~= e5m2 bits of exp(s*scale); e5m2 chosen so
a +-6 sigma logit can't reach the NaN encoding). Z rides as a ones-row in
the DoubleRow weights; 1/Z via scalar Ln->Exp(-x), staggered over the
following steps. GroupNorm rstd via exp(-0.5*ln(var+eps)). Residual adds and
broadcasts run on GpSimd. Conv weights are rstd-folded on device so PE conv
work starts without waiting for GN statistics to be applied to x. The first
two PV steps of each block are emitted together one step late so the psum
accumulator handoff never stalls on the previous block's drain.
"""
import numpy as np
import ml_dtypes

import concourse.bass as bass
import concourse.bacc as bacc
import concourse.tile as tile
from concourse import mybir
from concourse import bass_utils
from concourse.bass_interp import get_hw_module

F32 = mybir.dt.float32
BF16 = mybir.dt.bfloat16
FP8E4 = mybir.dt.float8e4
FP8E5 = mybir.dt.float8e5
U8 = mybir.dt.uint8
BF = ml_dtypes.bfloat16

B, C, Himg, Wimg = 4, 256, 64, 64
T = Himg * Wimg            # 4096 tokens
HEADS, D = 4, 64           # 4 heads x 64 dims
GROUPS, GS = 32, 8         # groupnorm: 32 groups of 8 channels
EPS = 1e-5
TQ = T // 2                # queries per core (2048)
NTT = TQ // 512            # query tiles of 512
NSC = T // 128             # 128-key chunks (32)
NS2 = T // 256             # 256-key PV steps (16)
SCALE = 1.0 / np.sqrt(D)
LOG2E = float(np.log2(np.e))
A5 = float(4.0 * LOG2E * SCALE)     # schraudolph mult (e5m2 bits)
B5 = float(60.0 - 0.24)             # schraudolph bias (weighted-rms centering)
Exp = mybir.ActivationFunctionType.Exp
Ln = mybir.ActivationFunctionType.Ln
Ident = mybir.ActivationFunctionType.Identity
DR = mybir.MatmulPerfMode.DoubleRow

_CACHED = {}


def _patch_act_tables():
    """Restrict the act-table chooser to natural_log_exp_and_others so the
    scalar engine never reloads tables (exp+ln live in one set; identity
    copies are expressed as Identity, also in that set). Set order is
    preserved so act_func_set_id stays aligned with act_info.json."""
    if getattr(bacc, "_act_tables_patched", False):
        return
    orig = bacc.get_activation_tables

    def patched(arch):
        t = orig(arch)
        return {name: (fns if name == "natural_log_exp_and_others" else set())
                for name, fns in t.items()}

    bacc.get_activation_tables = patched
    bacc._act_tables_patched = True


def _build():
    _patch_act_tables()
    nc = bacc.Bacc("TRN2", target_bir_lowering=False, debug=False,
                   enable_asserts=False, num_devices=8)

    xb_d = nc.dram_tensor("xb", [C, T], BF16, kind="ExternalInput")
    x8_d = nc.dram_tensor("x8", [128, 2, T], FP8E4, kind="ExternalInput")
    qkvT_d = nc.dram_tensor("qkvT", [C, 3 * C], BF16, kind="ExternalInput")
    qkvb_d = nc.dram_tensor("qkvb", [3 * C, 1], F32, kind="ExternalInput")
    projT_d = nc.dram_tensor("projT", [HEADS, D, C], BF16, kind="ExternalInput")
    projb_d = nc.dram_tensor("projb", [C, 1], F32, kind="ExternalInput")
    gmat_d = nc.dram_tensor("gmat", [128, 16], F32, kind="ExternalInput")
    gmatT_d = nc.dram_tensor("gmatT", [16, 128], F32, kind="ExternalInput")
    out_d = nc.dram_tensor("out", [C, TQ], F32, kind="ExternalOutput")

    with tile.TileContext(nc) as tc:
        with (
            tc.tile_pool(name="consts", bufs=1) as consts,
            tc.tile_pool(name="data", bufs=1) as data,
            tc.tile_pool(name="gn", bufs=1) as gn,
            tc.tile_pool(name="pt", bufs=10) as ppool,
            tc.tile_pool(name="dn", bufs=2) as dn,
            tc.tile_pool(name="ao", bufs=4) as ao,
            tc.tile_pool(name="ah", bufs=1) as ahpool,
            tc.tile_pool(name="ps", bufs=3, space="PSUM") as psum_s,
            tc.tile_pool(name="pa", bufs=1, space="PSUM") as psum_a,
        ):
            # ---------------- tiles ----------------
            qkvT2 = consts.tile([128, 2, 3 * C], BF16, tag="qkvT2", name="qkvT2")
            qkvT_sb = [qkvT2[:, ct, :] for ct in range(2)]
            projT2p = consts.tile([128, 2, C], BF16, tag="projT2p", name="projT2p")
            b8 = consts.tile([128, 8], F32, tag="b8", name="b8")
            b_in = [b8[:, m:m + 1] for m in range(6)]
            pb_sb = [b8[:, 6 + oc:7 + oc] for oc in range(2)]
            gmat_sb = consts.tile([128, 16], F32, tag="gmat", name="gmat")
            gmatT_sb = consts.tile([16, 128], F32, tag="gmatT", name="gmatT")
            eps_t = gn.tile([16, 1], F32, tag="eps", name="eps")
            nc.vector.memset(eps_t[:], EPS)
            # preload the exp/ln act table while DMAs are in flight
            warm = gn.tile([16, 1], F32, tag="warm", name="warm")
            nc.scalar.activation(warm[:], eps_t[:], Exp)

            xb_sb, st_sb = [], []
            for ct in range(2):
                xt = data.tile([128, T], BF16, tag=f"xb{ct}", name=f"xb{ct}")
                xb_sb.append(xt)
                st = gn.tile([128, 8, 6], F32, tag=f"st{ct}", name=f"st{ct}")
                st_sb.append(st)
            # fp8 copy of x in conv DoubleRow pair layout (ct pairs on the
            # free dim), host-converted
            x8 = data.tile([128, 2, T], FP8E4, tag="x8", name="x8")

            # ---------------- loads: x chunks first, weights woven -------
            qdma = [nc.sync, nc.scalar, nc.gpsimd]
            for c8 in range(8):
                for ct in range(2):
                    sl = slice(c8 * 512, (c8 + 1) * 512)
                    qdma[(2 * c8 + ct) % 3].dma_start(
                        xb_sb[ct][:, sl], xb_d.ap()[ct * 128:(ct + 1) * 128, sl])
            nc.scalar.dma_start(qkvT2[:], qkvT_d.ap()[:].rearrange(
                "(ct p) o -> p ct o", ct=2))
            nc.sync.dma_start(b8[:, 0:6], qkvb_d.ap()[:].rearrange(
                "(m p) one -> p (m one)", m=6))
            nc.sync.dma_start(b8[:, 6:8], projb_d.ap()[:].rearrange(
                "(oc p) one -> p (oc one)", oc=2))
            nc.sync.dma_start(projT2p[:], projT_d.ap()[:].rearrange(
                "(p2 u) d o -> (u d) p2 o", p2=2))
            nc.gpsimd.dma_start(gmat_sb[:], gmat_d.ap()[:])
            nc.gpsimd.dma_start(gmatT_sb[:], gmatT_d.ap()[:])
            # fp8 x behind the bf16 x (needed only once convs start)
            for c8 in range(8):
                sl = slice(c8 * 512, (c8 + 1) * 512)
                qdma[c8 % 3].dma_start(x8[:, :, sl], x8_d.ap()[:, :, sl])
            # PE warm-up: junk matmuls on the first landed chunk keep the
            # HAM activity monitor busy so the real pipeline starts at
            # 2.4GHz instead of cold 1.2GHz.
            junk_ps = psum_s.tile([128, 512], F32, tag="ps", name="ps")
            for _ in range(32):
                nc.tensor.matmul(junk_ps[:], xb_sb[0][0:128, 0:128],
                                 xb_sb[0][:, 0:512], start=True, stop=True)
            for c4 in range(4):
                for ct in range(2):
                    for half in range(2):
                        sh = slice(c4 * 1024 + half * 512,
                                   c4 * 1024 + (half + 1) * 512)
                        nc.vector.bn_stats(st_sb[ct][:, 2 * c4 + half, :],
                                           xb_sb[ct][:, sh])
            # ---------------- GroupNorm statistics ----------------
            # ct0 assembly on DVE, ct1 on GpSimd: the two serial chains
            # overlap instead of queueing on one engine
            stats2 = []
            for ct in range(2):
                eng = nc.vector if ct == 0 else nc.gpsimd
                mv = gn.tile([128, 2], F32, tag=f"mv{ct}", name=f"mv{ct}")
                nc.vector.bn_aggr(mv[:], st_sb[ct][:])
                s2 = gn.tile([128, 2], F32, tag=f"s2{ct}", name=f"s2{ct}")
                eng.tensor_copy(s2[:, 0:1], mv[:, 0:1])
                m2 = gn.tile([128, 1], F32, tag=f"m2{ct}", name=f"m2{ct}")
                eng.tensor_mul(m2[:], mv[:, 0:1], mv[:, 0:1])
                eng.tensor_add(s2[:, 1:2], m2[:], mv[:, 1:2])
                stats2.append(s2)

            # group (mean, E[x^2]) -> per-group rstd via exp(-0.5*ln(var+eps))
            gs_ps, bc_sb = [], []
            vg = gn.tile([16, 2], F32, tag="vg", name="vg")
            for ct in range(2):
                eng = nc.vector if ct == 0 else nc.gpsimd
                g1 = psum_s.tile([16, 2], F32, tag="ps", name="ps")
                nc.tensor.matmul(g1[:], gmat_sb[:], stats2[ct][:],
                                 start=True, stop=True)
                gsb = gn.tile([16, 2], F32, tag=f"gsb{ct}", name=f"gsb{ct}")
                nc.vector.tensor_copy(gsb[:], g1[:])
                gs_ps.append(gsb)
                m2g = gn.tile([16, 1], F32, tag=f"m2g{ct}", name=f"m2g{ct}")
                eng.tensor_mul(m2g[:], gsb[:, 0:1], gsb[:, 0:1])
                eng.tensor_sub(vg[:, ct:ct + 1], gsb[:, 1:2], m2g[:])
            for _ in range(6):
                nc.tensor.matmul(junk_ps[:], xb_sb[0][0:128, 0:128],
                                 xb_sb[0][:, 0:512], start=True, stop=True)
            lgv = gn.tile([16, 2], F32, tag="lgv", name="lgv")
            nc.scalar.activation(lgv[:], vg[:], Ln, bias=eps_t[:])
            rg = gn.tile([16, 2], F32, tag="rg", name="rg")
            nc.scalar.activation(rg[:], lgv[:], Exp, scale=-0.5)
            for ct in range(2):
                bcv = gn.tile([16, 2], F32, tag=f"bcv{ct}", name=f"bcv{ct}")
                nc.vector.tensor_copy(bcv[:, 0:1], gs_ps[ct][:, 0:1])
                nc.vector.tensor_copy(bcv[:, 1:2], rg[:, ct:ct + 1])
                b1 = psum_s.tile([128, 2], F32, tag="ps", name="ps")
                nc.tensor.matmul(b1[:], gmatT_sb[:], bcv[:],
                                 start=True, stop=True)
                bsb = gn.tile([128, 2], F32, tag=f"bc{ct}", name=f"bc{ct}")
                nc.vector.tensor_copy(bsb[:], b1[:])
                bc_sb.append(bsb)

            for _ in range(8):
                nc.tensor.matmul(junk_ps[:], xb_sb[0][0:128, 0:128],
                                 xb_sb[0][:, 0:512], start=True, stop=True)
            # ------- fold rstd into weights (bf16 for the mu path, fp8 for
            # the convs); bias b2 = b - W'mu -------
            qkvS_sb, mu_bf = [], []
            ws8 = consts.tile([128, 2, 3 * C], FP8E4, tag="ws8", name="ws8")
            for ct in range(2):
                eng = nc.vector if ct == 0 else nc.gpsimd
                ws = consts.tile([128, 3 * C], BF16, tag=f"qkvS{ct}", name=f"qkvS{ct}")
                eng.tensor_scalar_mul(out=ws[:], in0=qkvT_sb[ct][:],
                                      scalar1=bc_sb[ct][:, 1:2])
                qkvS_sb.append(ws)
                nc.vector.tensor_scalar_mul(out=ws8[:, ct, :], in0=qkvT_sb[ct][:],
                                            scalar1=bc_sb[ct][:, 1:2])
                mb = gn.tile([128, 1], BF16, tag=f"mub{ct}", name=f"mub{ct}")
                eng.tensor_copy(mb[:], bc_sb[ct][:, 0:1])
                mu_bf.append(mb)
            b2_sb = [None] * 6
            for m in (2, 3, 0, 1, 4, 5):     # k biases first: K(0,0) gates S
                wm = psum_s.tile([128, 1], F32, tag="ps", name="ps")
                for ct in range(2):
                    nc.tensor.matmul(wm[:], qkvS_sb[ct][:, m * 128:(m + 1) * 128],
                                     mu_bf[ct][:], start=(ct == 0), stop=(ct == 1))
                b2 = gn.tile([128, 1], F32, tag=f"b2_{m}", name=f"b2_{m}")
                nc.vector.tensor_sub(b2[:], b_in[m][:], wm[:])
                b2_sb[m] = b2
            # bf16 copies of the v-slice biases for the proj-bias fold;
            # head pair (2*p2, 2*p2+1) sits at partitions 0-63 / 64-127.
            bv_p2 = []
            for m in (4, 5):
                bb = gn.tile([128, 1], BF16, tag=f"bv{m}", name=f"bv{m}")
                nc.vector.tensor_copy(bb[:], b2_sb[m][:])
                bv_p2.append(bb)

            # ---------------- SBUF destination tiles ----------------
            k_sb = [data.tile([128, T], BF16, tag=f"k{p}", name=f"k{p}")
                    for p in range(2)]
            q_sb = [data.tile([128, TQ], BF16, tag=f"q{p}", name=f"q{p}")
                    for p in range(2)]
            # V in fp8e4, PV-DoubleRow layout: (s2, h, j, c) with c-stride 80
            # (j-step %16==0 per the s3_lw dual-fp8 rule); c=64 is the ones
            # row that accumulates Z.
            vt_sb = data.tile([128, NS2 * HEADS * 2 * 80], FP8E4,
                              tag="vt", name="vt")
            vt5 = vt_sb[:].rearrange("p (s h j c) -> p s h j c",
                                     s=NS2, h=HEADS, j=2, c=80)
            nc.vector.memset(vt5[:, :, :, :, 64:65], 1.0)

            # pb2[oc] = projb[oc] + sum_h projT_h[:,oc]^T @ bv_h
            pb2_sb = []

            def emit_pb2():
                for oc in range(2):
                    pv = psum_s.tile([128, 1], F32, tag="ps", name="ps")
                    for p2 in range(2):
                        nc.tensor.matmul(pv[:],
                                         projT2p[:, p2, oc * 128:(oc + 1) * 128],
                                         bv_p2[p2][:], start=(p2 == 0), stop=(p2 == 1))
                    pb2 = gn.tile([128, 1], F32, tag=f"pb2_{oc}", name=f"pb2_{oc}")
                    nc.vector.tensor_add(pb2[:], pb_sb[oc][:], pv[:])
                    pb2_sb.append(pb2)

            # ------------- conv units (fp8 DoubleRow, woven in) ----------
            def emit_k(p, t8):
                kv = psum_s.tile([128, 512], F32, tag="ps", name="ps")
                nc.tensor.matmul(
                    kv[:], ws8[:, :, C + p * 128:C + (p + 1) * 128],
                    x8[:, :, t8 * 512:(t8 + 1) * 512],
                    start=True, stop=True, perf_mode=DR)
                # bias add on ScalarE: Lrelu(alpha=1) == identity, in-table
                nc.scalar.activation(
                    k_sb[p][:, t8 * 512:(t8 + 1) * 512], kv[:],
                    Ident, bias=b2_sb[2 + p][:])

            def emit_q(p, t4, qeng="s"):
                qp = psum_s.tile([128, 512], F32, tag="ps", name="ps")
                nc.tensor.matmul(
                    qp[:], ws8[:, :, p * 128:(p + 1) * 128],
                    x8[:, :, t4 * 512:(t4 + 1) * 512],
                    start=True, stop=True, perf_mode=DR)
                if qeng == "s":
                    nc.scalar.activation(
                        q_sb[p][:, t4 * 512:(t4 + 1) * 512], qp[:],
                        Ident, bias=b2_sb[p][:])
                else:
                    nc.vector.tensor_scalar_add(
                        out=q_sb[p][:, t4 * 512:(t4 + 1) * 512],
                        in0=qp[:], scalar1=b2_sb[p][:])

            def emit_vt(i, eng):
                vp = psum_s.tile([128, C], F32, tag="ps", name="ps")
                nc.tensor.matmul(
                    vp[:], x8[:, :, i * 128:(i + 1) * 128],
                    ws8[:, :, 2 * C:3 * C],
                    start=True, stop=True, perf_mode=DR)
                dst = vt5[:, i // 2, :, i % 2, 0:64]
                if eng == "s":
                    nc.scalar.copy(dst, vp[:].rearrange("p (h c) -> p h c", c=64))
                else:
                    nc.vector.tensor_copy(dst, vp[:].rearrange("p (h c) -> p h c", c=64))

            # per-(block, chunk) pre-emit schedule of conv units
            sched = {}

            def add_sched(b, i, fn):
                sched.setdefault((b, i), []).append(fn)

            add_sched(0, 0, lambda: emit_vt(0, "s"))
            add_sched(0, 0, lambda: emit_vt(1, "v"))
            add_sched(0, 0, lambda: emit_vt(2, "s"))
            for j in range(3, 32):
                b0c = (j - 3) // 2 + 1          # chunks 1..15, two vts per chunk
                add_sched(0, b0c, (lambda jj: lambda: emit_vt(
                    jj, "s" if jj % 2 else "v"))(j))
            for t8 in range(1, 8):
                add_sched(0, t8, (lambda t: lambda: emit_k(0, t))(t8))
            for t4 in range(1, 4):
                add_sched(t4 - 1, 20, (lambda t: lambda: emit_q(0, t))(t4))
            for t8 in range(8):
                add_sched(1 + t8 // 3, 8 + 3 * (t8 % 3), (lambda t: lambda: emit_k(1, t))(t8))
            for t4 in range(4):
                add_sched(2, 17 + 3 * t4, (lambda t: lambda: emit_q(1, t))(t4))
            add_sched(0, 16, emit_pb2)

            # ---------------- attention ----------------
            # 128-step pipeline (8 blocks x 16 key steps of 256). Per step:
            # two S sub-chunks (128 keys each) + their exps (fp8e5 out), with
            # the lagged PV DoubleRow matmuls interleaved between them so
            # every LDWEIGHTS hides behind the previous stream. Block's PV
            # steps 0+1 are emitted together at the step-1 slot so the a_ps
            # handoff never stalls on the previous block's drain.
            PV_LAG = 7
            ah_sb = {}
            post = {}          # g -> list of closures, run before step g

            def at(g, fn):
                post.setdefault(g, []).append(fn)

            def emit_pv_u(pblk, s2, u, p_t):
                p = pblk // NTT
                h = 2 * p + u
                rhs = p_t[:, u * 1024:(u + 1) * 1024].rearrange(
                    "p (j n) -> p j n", j=2)
                nc.tensor.matmul(
                    a_cur[pblk][:, u * 512:(u + 1) * 512],
                    vt5[:, s2, h, :, 0:65], rhs,
                    start=(s2 == 0), stop=(s2 == NS2 - 1), perf_mode=DR)

            def emit_drain(blk, g):
                p, tt = blk // NTT, blk % NTT
                a_ps = a_cur.pop(blk)
                araw = dn.tile([65, 1024], BF16, tag="araw", name="araw")
                last = (blk == 2 * NTT - 1)
                nc.scalar.copy(araw[:, 0:512], a_ps[:, 0:512])
                if last:
                    nc.vector.tensor_copy(araw[:, 512:1024], a_ps[:, 512:1024])
                else:
                    at(g + 1, lambda: nc.vector.tensor_copy(
                        araw[:, 512:1024], a_ps[:, 512:1024]))

                if last:
                    # pipelined per-half 1/Z off the raw psum accumulator:
                    # scalar/gpsimd/vector FIFOs overlap the halves
                    def fin_last():
                        zl = dn.tile([1, 1024], F32, tag="zl", name="zl")
                        zi = dn.tile([1, 1024], F32, tag="zi", name="zi")
                        d_bc = dn.tile([64, 1024], F32, tag="dbc", name="dbc")
                        ah = ahpool.tile([128, 512], BF16, tag=f"ah{p}_{tt}",
                                         name=f"ah{p}_{tt}")
                        for u in range(2):
                            hs = slice(u * 512, (u + 1) * 512)
                            nc.scalar.activation(zl[:, hs], a_ps[64:65, hs], Ln)
                            nc.scalar.activation(zi[:, hs], zl[:, hs], Exp,
                                                 scale=-1.0)
                            nc.gpsimd.partition_broadcast(d_bc[:, hs], zi[:, hs])
                            nc.vector.tensor_mul(ah[u * 64:(u + 1) * 64, :],
                                                 araw[0:64, hs], d_bc[:, hs])
                        ah_sb[(p, tt)] = ah
                    at(g + 1, fin_last)
                    at(g + 2, make_proj(tt))
                    return

                def ln():
                    zl = dn.tile([1, 1024], F32, tag="zl", name="zl")
                    nc.scalar.activation(zl[:], araw[64:65, :], Ln)

                    def inv_bc():
                        zi = dn.tile([1, 1024], F32, tag="zi", name="zi")
                        nc.scalar.activation(zi[:], zl[:], Exp, scale=-1.0)
                        d_bc = dn.tile([64, 1024], F32, tag="dbc", name="dbc")
                        nc.gpsimd.partition_broadcast(d_bc[:], zi[:])

                        def ah_fin():
                            ah = ahpool.tile([128, 512], BF16, tag=f"ah{p}_{tt}",
                                             name=f"ah{p}_{tt}")
                            for u in range(2):
                                nc.gpsimd.tensor_mul(
                                    ah[u * 64:(u + 1) * 64, :],
                                    araw[0:64, u * 512:(u + 1) * 512],
                                    d_bc[:, u * 512:(u + 1) * 512])
                            ah_sb[(p, tt)] = ah
                        at(g + 7, ah_fin)
                    at(g + 4, inv_bc)
                at(g + 2, ln)
                if p == 1:
                    at(g + 9, make_proj(tt))

            def make_proj(tt):
                def proj():
                    for oc in range(2):
                        pr = psum_s.tile([128, 512], F32, tag="ps", name="ps")
                        for p2 in range(2):
                            nc.tensor.matmul(
                                pr[:], projT2p[:, p2, oc * 128:(oc + 1) * 128],
                                ah_sb[(p2, tt)][:],
                                start=(p2 == 0), stop=(p2 == 1))
                        o1 = ao.tile([128, 512], F32, tag="o1", name="o1")
                        nc.vector.tensor_scalar_add(out=o1[:], in0=pr[:],
                                                    scalar1=pb2_sb[oc][:])
                        o2 = ao.tile([128, 512], F32, tag="o2", name="o2")
                        o2eng = nc.vector if tt == NTT - 1 else nc.gpsimd
                        o2eng.tensor_add(o2[:], o1[:],
                                         xb_sb[oc][:, tt * 512:(tt + 1) * 512])
                        nc.sync.dma_start(
                            out_d.ap()[oc * 128:(oc + 1) * 128,
                                       tt * 512:(tt + 1) * 512], o2[:])
                return proj

            emit_k(0, 0)
            emit_q(0, 0, qeng="v")

            NG = 2 * NTT * NS2          # 128 global steps
            pts = {}
            a_cur = {}

            def pv_units_for(g):
                gp = g - PV_LAG
                if gp < 0:
                    return []
                pblk, ps2 = gp // NS2, gp % NS2
                if ps2 == 0:
                    return []
                units = []
                if ps2 == 1:
                    units += [(pblk, 0, 0), (pblk, 0, 1)]
                units += [(pblk, ps2, 0), (pblk, ps2, 1)]
                return units

            def run_pv_units(units):
                for pblk, ps2, u in units:
                    if pblk not in a_cur:
                        a_cur[pblk] = psum_a.tile([65, 1024], F32,
                                                  tag="pa", name="pa")
                    emit_pv_u(pblk, ps2, u, pts[(pblk, ps2)])
                    if u == 1:
                        pts.pop((pblk, ps2))

            for g in range(NG + PV_LAG + 1):
                for fn in post.pop(g, ()):
                    fn()
                units = pv_units_for(g)
                if g < NG:
                    blk, s2 = g // NS2, g % NS2
                    p, tt = blk // NTT, blk % NTT
                    p_t = ppool.tile([128, 2048], FP8E5, tag="pt", name="pt")
                    p_t4 = p_t[:].rearrange("p (u j n) -> p u j n", u=2, j=2)
                    pts[(blk, s2)] = p_t
                    for j in range(2):
                        i = 2 * s2 + j
                        for fn in sched.pop((blk, i), ()):
                            fn()
                        s_ps = psum_s.tile([128, 1024], F32, tag="ps", name="ps")
                        for u in range(2):
                            nc.tensor.matmul(
                                s_ps[:, u * 512:(u + 1) * 512],
                                k_sb[p][u * 64:(u + 1) * 64, i * 128:(i + 1) * 128],
                                q_sb[p][u * 64:(u + 1) * 64, tt * 512:(tt + 1) * 512],
                                start=True, stop=True,
                                tile_position=(u * 64, 0))
                        dst = p_t4[:, :, j, :]
                        src = s_ps[:].rearrange("p (u n) -> p u n", u=2)
                        if j == 0:
                            nc.scalar.activation(dst, src, Exp,
                                                 scale=float(SCALE))
                        else:
                            nc.vector.tensor_scalar(
                                out=dst.bitcast(U8), in0=src,
                                scalar1=A5, scalar2=B5,
                                op0=mybir.AluOpType.mult,
                                op1=mybir.AluOpType.add)
                    run_pv_units(units)
                else:
                    run_pv_units(units)
                gp = g - PV_LAG
                if gp >= 0 and gp % NS2 == NS2 - 1:
                    emit_drain(gp // NS2, g)
            for g in range(NG + PV_LAG + 1, NG + PV_LAG + 12):
                for fn in post.pop(g, ()):
                    fn()
            assert not post and not sched and not pts

    nc.compile()
    nc.m = get_hw_module(nc.m)
    return nc


def _host_prep(inputs):
    x = np.asarray(inputs["x"], np.float32)
    gn_w = np.asarray(inputs["gn_weight"], np.float32)
    gn_b = np.asarray(inputs["gn_bias"], np.float32)
    qkv_w = np.asarray(inputs["qkv_w"], np.float32)
    qkv_b = np.asarray(inputs["qkv_b"], np.float32)
    proj_w = np.asarray(inputs["proj_w"], np.float32)
    proj_b = np.asarray(inputs["proj_b"], np.float32)

    W_ = qkv_w * gn_w[None, :]
    b_ = qkv_w @ gn_b + qkv_b
    qkvT = np.ascontiguousarray(W_.T).astype(BF)
    projT = np.ascontiguousarray(proj_w.T.reshape(HEADS, D, C)).astype(BF)

    gmat = np.zeros((128, 16), np.float32)
    gmatT = np.zeros((16, 128), np.float32)
    for ch in range(128):
        gmat[ch, ch // GS] = 1.0 / GS
        gmatT[ch // GS, ch] = 1.0
    shared = {
        "qkvT": qkvT,
        "qkvb": b_.reshape(3 * C, 1).astype(np.float32),
        "projT": projT,
        "projb": proj_b.reshape(C, 1).astype(np.float32),
        "gmat": gmat,
        "gmatT": gmatT,
    }
    x3 = x.reshape(B, C, T).astype(BF)
    in_maps = []
    for j in range(8):
        b, hf = j // 2, j % 2
        m = dict(shared)
        if hf == 0:
            m["xb"] = np.ascontiguousarray(x3[b])
        else:
            m["xb"] = np.ascontiguousarray(
                np.concatenate([x3[b][:, TQ:], x3[b][:, :TQ]], axis=1))
        m["x8"] = np.ascontiguousarray(
            m["xb"].reshape(2, 128, T).transpose(1, 0, 2)).astype(
                ml_dtypes.float8_e4m3fn)
        in_maps.append(m)
    return x, in_maps


def kernel(**inputs) -> np.ndarray:
    if "nc" not in _CACHED:
        _CACHED["nc"] = _build()
    nc = _CACHED["nc"]
    x, in_maps = _host_prep(inputs)
    res = bass_utils.run_bass_kernel_spmd(nc, in_maps, core_ids=list(range(8)))
    out = np.zeros((B, C, T), np.float32)
    for j in range(8):
        b, hf = j // 2, j % 2
        out[b][:, hf * TQ:(hf + 1) * TQ] = np.asarray(
            res.results[j]["out"], np.float32)
    return out.reshape(B, C, Himg, Wimg)


# revision 28
# speedup vs baseline: 1.0379x; 1.0379x over previous
"""AttentionBlock (GroupNorm -> qkv conv1x1 -> 4-head attention -> proj -> residual)
as a distributed Bass/Tile kernel on 8 TRN2 NeuronCores.

Sharding: core j handles batch b = j//2 and query-half h = j%2. The host
permutes x's spatial columns per core so queries are always cols 0:2048
(attention is permutation-invariant over keys). K/V are computed full-length
per core, so output slices are disjoint and no collectives are needed.

PV and the qkv convs run in fp8 DoubleRow (contraction = 2x128 per pass at
0.5 cyc/col): V is stored e4m3 with the per-head k-pair layout [128, 2, 80]
(j-stride %16==0 per the s3_lw dual-fp8 ISA rule), probabilities are e5m2,
conv weights/inputs e4m3 (x8 derived on device from the bf16 x). Softmax exp
is split between ScalarE (real Exp, fp8e5 out) and VectorE (Schraudolph
bit-trick: uint8 round of a*s+b ~= e5m2 bits of exp(s*scale); e5m2 chosen so
a +-6 sigma logit can't reach the NaN encoding). Z rides as a ones-row in
the DoubleRow weights; 1/Z via scalar Ln->Exp(-x), staggered over the
following steps. GroupNorm rstd via exp(-0.5*ln(var+eps)). Residual adds and
broadcasts run on GpSimd. Conv weights are rstd-folded on device so PE conv
work starts without waiting for GN statistics to be applied to x. The first
two PV steps of each block are emitted together one step late so the psum
accumulator handoff never stalls on the previous block's drain.
"""
import numpy as np
import ml_dtypes

import concourse.bass as bass
import concourse.bacc as bacc
import concourse.tile as tile
from concourse import mybir
from concourse import bass_utils
from concourse.bass_interp import get_hw_module

F32 = mybir.dt.float32
BF16 = mybir.dt.bfloat16
FP8E4 = mybir.dt.float8e4
FP8E5 = mybir.dt.float8e5
U8 = mybir.dt.uint8
BF = ml_dtypes.bfloat16

B, C, Himg, Wimg = 4, 256, 64, 64
T = Himg * Wimg            # 4096 tokens
HEADS, D = 4, 64           # 4 heads x 64 dims
GROUPS, GS = 32, 8         # groupnorm: 32 groups of 8 channels
EPS = 1e-5
TQ = T // 2                # queries per core (2048)
NTT = TQ // 512            # query tiles of 512
NSC = T // 128             # 128-key chunks (32)
NS2 = T // 256             # 256-key PV steps (16)
SCALE = 1.0 / np.sqrt(D)
LOG2E = float(np.log2(np.e))
A5 = float(4.0 * LOG2E * SCALE)     # schraudolph mult (e5m2 bits)
B5 = float(60.0 - 0.24)             # schraudolph bias (weighted-rms centering)
Exp = mybir.ActivationFunctionType.Exp
Ln = mybir.ActivationFunctionType.Ln
Ident = mybir.ActivationFunctionType.Identity
DR = mybir.MatmulPerfMode.DoubleRow

_CACHED = {}


def _patch_act_tables():
    """Restrict the act-table chooser to natural_log_exp_and_others so the
    scalar engine never reloads tables (exp+ln live in one set; identity
    copies are expressed as Identity, also in that set). Set order is
    preserved so act_func_set_id stays aligned with act_info.json."""
    if getattr(bacc, "_act_tables_patched", False):
        return
    orig = bacc.get_activation_tables

    def patched(arch):
        t = orig(arch)
        return {name: (fns if name == "natural_log_exp_and_others" else set())
                for name, fns in t.items()}

    bacc.get_activation_tables = patched
    bacc._act_tables_patched = True


def _build():
    _patch_act_tables()
    nc = bacc.Bacc("TRN2", target_bir_lowering=False, debug=False,
                   enable_asserts=False, num_devices=8)

    xb_d = nc.dram_tensor("xb", [C, T], BF16, kind="ExternalInput")
    x8_d = nc.dram_tensor("x8", [128, 2, T], FP8E4, kind="ExternalInput")
    qkvT_d = nc.dram_tensor("qkvT", [C, 3 * C], BF16, kind="ExternalInput")
    qkvb_d = nc.dram_tensor("qkvb", [3 * C, 1], F32, kind="ExternalInput")
    projT_d = nc.dram_tensor("projT", [HEADS, D, C], BF16, kind="ExternalInput")
    projb_d = nc.dram_tensor("projb", [C, 1], F32, kind="ExternalInput")
    gmat_d = nc.dram_tensor("gmat", [128, 16], F32, kind="ExternalInput")
    gmatT_d = nc.dram_tensor("gmatT", [16, 128], F32, kind="ExternalInput")
    out_d = nc.dram_tensor("out", [C, TQ], F32, kind="ExternalOutput")

    with tile.TileContext(nc) as tc:
        with (
            tc.tile_pool(name="consts", bufs=1) as consts,
            tc.tile_pool(name="data", bufs=1) as data,
            tc.tile_pool(name="gn", bufs=1) as gn,
            tc.tile_pool(name="pt", bufs=10) as ppool,
            tc.tile_pool(name="dn", bufs=2) as dn,
            tc.tile_pool(name="ao", bufs=4) as ao,
            tc.tile_pool(name="ah", bufs=1) as ahpool,
            tc.tile_pool(name="ps", bufs=3, space="PSUM") as psum_s,
            tc.tile_pool(name="pa", bufs=1, space="PSUM") as psum_a,
        ):
            # ---------------- tiles ----------------
            qkvT2 = consts.tile([128, 2, 3 * C], BF16, tag="qkvT2", name="qkvT2")
            qkvT_sb = [qkvT2[:, ct, :] for ct in range(2)]
            projT2p = consts.tile([128, 2, C], BF16, tag="projT2p", name="projT2p")
            b8 = consts.tile([128, 8], F32, tag="b8", name="b8")
            b_in = [b8[:, m:m + 1] for m in range(6)]
            pb_sb = [b8[:, 6 + oc:7 + oc] for oc in range(2)]
            gmat_sb = consts.tile([128, 16], F32, tag="gmat", name="gmat")
            gmatT_sb = consts.tile([16, 128], F32, tag="gmatT", name="gmatT")
            eps_t = gn.tile([16, 1], F32, tag="eps", name="eps")
            nc.vector.memset(eps_t[:], EPS)
            # preload the exp/ln act table while DMAs are in flight
            warm = gn.tile([16, 1], F32, tag="warm", name="warm")
            nc.scalar.activation(warm[:], eps_t[:], Exp)

            xb_sb, st_sb = [], []
            for ct in range(2):
                xt = data.tile([128, T], BF16, tag=f"xb{ct}", name=f"xb{ct}")
                xb_sb.append(xt)
                st = gn.tile([128, 8, 6], F32, tag=f"st{ct}", name=f"st{ct}")
                st_sb.append(st)
            # fp8 copy of x in conv DoubleRow pair layout (ct pairs on the
            # free dim), host-converted
            x8 = data.tile([128, 2, T], FP8E4, tag="x8", name="x8")

            # ---------------- loads: x chunks first, weights woven -------
            qdma = [nc.sync, nc.scalar, nc.gpsimd]
            for c8 in range(8):
                for ct in range(2):
                    sl = slice(c8 * 512, (c8 + 1) * 512)
                    qdma[(2 * c8 + ct) % 3].dma_start(
                        xb_sb[ct][:, sl], xb_d.ap()[ct * 128:(ct + 1) * 128, sl])
            nc.scalar.dma_start(qkvT2[:], qkvT_d.ap()[:].rearrange(
                "(ct p) o -> p ct o", ct=2))
            nc.sync.dma_start(b8[:, 0:6], qkvb_d.ap()[:].rearrange(
                "(m p) one -> p (m one)", m=6))
            nc.sync.dma_start(b8[:, 6:8], projb_d.ap()[:].rearrange(
                "(oc p) one -> p (oc one)", oc=2))
            nc.sync.dma_start(projT2p[:], projT_d.ap()[:].rearrange(
                "(p2 u) d o -> (u d) p2 o", p2=2))
            nc.gpsimd.dma_start(gmat_sb[:], gmat_d.ap()[:])
            nc.gpsimd.dma_start(gmatT_sb[:], gmatT_d.ap()[:])
            # fp8 x behind the bf16 x (needed only once convs start)
            for c8 in range(8):
                sl = slice(c8 * 512, (c8 + 1) * 512)
                qdma[c8 % 3].dma_start(x8[:, :, sl], x8_d.ap()[:, :, sl])
            # PE warm-up: junk matmuls on the first landed chunk keep the
            # HAM activity monitor busy so the real pipeline starts at
            # 2.4GHz instead of cold 1.2GHz.
            junk_ps = psum_s.tile([128, 512], F32, tag="ps", name="ps")
            for _ in range(32):
                nc.tensor.matmul(junk_ps[:], xb_sb[0][0:128, 0:128],
                                 xb_sb[0][:, 0:512], start=True, stop=True)
            for c4 in range(4):
                for ct in range(2):
                    for half in range(2):
                        sh = slice(c4 * 1024 + half * 512,
                                   c4 * 1024 + (half + 1) * 512)
                        nc.vector.bn_stats(st_sb[ct][:, 2 * c4 + half, :],
                                           xb_sb[ct][:, sh])
            # ---------------- GroupNorm statistics ----------------
            stats2 = []
            for ct in range(2):
                mv = gn.tile([128, 2], F32, tag=f"mv{ct}", name=f"mv{ct}")
                nc.vector.bn_aggr(mv[:], st_sb[ct][:])
                s2 = gn.tile([128, 2], F32, tag=f"s2{ct}", name=f"s2{ct}")
                nc.vector.tensor_copy(s2[:, 0:1], mv[:, 0:1])
                m2 = gn.tile([128, 1], F32, tag=f"m2{ct}", name=f"m2{ct}")
                nc.vector.tensor_mul(m2[:], mv[:, 0:1], mv[:, 0:1])
                nc.vector.tensor_add(s2[:, 1:2], m2[:], mv[:, 1:2])
                stats2.append(s2)

            # group (mean, E[x^2]) -> per-group rstd via exp(-0.5*ln(var+eps))
            gs_ps, bc_sb = [], []
            vg = gn.tile([16, 2], F32, tag="vg", name="vg")
            for ct in range(2):
                g1 = psum_s.tile([16, 2], F32, tag="ps", name="ps")
                nc.tensor.matmul(g1[:], gmat_sb[:], stats2[ct][:],
                                 start=True, stop=True)
                gsb = gn.tile([16, 2], F32, tag=f"gsb{ct}", name=f"gsb{ct}")
                nc.vector.tensor_copy(gsb[:], g1[:])
                gs_ps.append(gsb)
                m2g = gn.tile([16, 1], F32, tag=f"m2g{ct}", name=f"m2g{ct}")
                nc.vector.tensor_mul(m2g[:], gsb[:, 0:1], gsb[:, 0:1])
                nc.vector.tensor_sub(vg[:, ct:ct + 1], gsb[:, 1:2], m2g[:])
            for _ in range(6):
                nc.tensor.matmul(junk_ps[:], xb_sb[0][0:128, 0:128],
                                 xb_sb[0][:, 0:512], start=True, stop=True)
            lgv = gn.tile([16, 2], F32, tag="lgv", name="lgv")
            nc.scalar.activation(lgv[:], vg[:], Ln, bias=eps_t[:])
            rg = gn.tile([16, 2], F32, tag="rg", name="rg")
            nc.scalar.activation(rg[:], lgv[:], Exp, scale=-0.5)
            for ct in range(2):
                bcv = gn.tile([16, 2], F32, tag=f"bcv{ct}", name=f"bcv{ct}")
                nc.vector.tensor_copy(bcv[:, 0:1], gs_ps[ct][:, 0:1])
                nc.vector.tensor_copy(bcv[:, 1:2], rg[:, ct:ct + 1])
                b1 = psum_s.tile([128, 2], F32, tag="ps", name="ps")
                nc.tensor.matmul(b1[:], gmatT_sb[:], bcv[:],
                                 start=True, stop=True)
                bsb = gn.tile([128, 2], F32, tag=f"bc{ct}", name=f"bc{ct}")
                nc.vector.tensor_copy(bsb[:], b1[:])
                bc_sb.append(bsb)

            for _ in range(8):
                nc.tensor.matmul(junk_ps[:], xb_sb[0][0:128, 0:128],
                                 xb_sb[0][:, 0:512], start=True, stop=True)
            # ------- fold rstd into weights (bf16 for the mu path, fp8 for
            # the convs); bias b2 = b - W'mu -------
            qkvS_sb, mu_bf = [], []
            ws8 = consts.tile([128, 2, 3 * C], FP8E4, tag="ws8", name="ws8")
            for ct in range(2):
                ws = consts.tile([128, 3 * C], BF16, tag=f"qkvS{ct}", name=f"qkvS{ct}")
                nc.vector.tensor_scalar_mul(out=ws[:], in0=qkvT_sb[ct][:],
                                            scalar1=bc_sb[ct][:, 1:2])
                qkvS_sb.append(ws)
                nc.vector.tensor_scalar_mul(out=ws8[:, ct, :], in0=qkvT_sb[ct][:],
                                            scalar1=bc_sb[ct][:, 1:2])
                mb = gn.tile([128, 1], BF16, tag=f"mub{ct}", name=f"mub{ct}")
                nc.vector.tensor_copy(mb[:], bc_sb[ct][:, 0:1])
                mu_bf.append(mb)
            b2_sb = [None] * 6
            for m in (2, 3, 0, 1, 4, 5):     # k biases first: K(0,0) gates S
                wm = psum_s.tile([128, 1], F32, tag="ps", name="ps")
                for ct in range(2):
                    nc.tensor.matmul(wm[:], qkvS_sb[ct][:, m * 128:(m + 1) * 128],
                                     mu_bf[ct][:], start=(ct == 0), stop=(ct == 1))
                b2 = gn.tile([128, 1], F32, tag=f"b2_{m}", name=f"b2_{m}")
                nc.vector.tensor_sub(b2[:], b_in[m][:], wm[:])
                b2_sb[m] = b2
            # bf16 copies of the v-slice biases for the proj-bias fold;
            # head pair (2*p2, 2*p2+1) sits at partitions 0-63 / 64-127.
            bv_p2 = []
            for m in (4, 5):
                bb = gn.tile([128, 1], BF16, tag=f"bv{m}", name=f"bv{m}")
                nc.vector.tensor_copy(bb[:], b2_sb[m][:])
                bv_p2.append(bb)

            # ---------------- SBUF destination tiles ----------------
            k_sb = [data.tile([128, T], BF16, tag=f"k{p}", name=f"k{p}")
                    for p in range(2)]
            q_sb = [data.tile([128, TQ], BF16, tag=f"q{p}", name=f"q{p}")
                    for p in range(2)]
            # V in fp8e4, PV-DoubleRow layout: (s2, h, j, c) with c-stride 80
            # (j-step %16==0 per the s3_lw dual-fp8 rule); c=64 is the ones
            # row that accumulates Z.
            vt_sb = data.tile([128, NS2 * HEADS * 2 * 80], FP8E4,
                              tag="vt", name="vt")
            vt5 = vt_sb[:].rearrange("p (s h j c) -> p s h j c",
                                     s=NS2, h=HEADS, j=2, c=80)
            nc.vector.memset(vt5[:, :, :, :, 64:65], 1.0)

            # pb2[oc] = projb[oc] + sum_h projT_h[:,oc]^T @ bv_h
            pb2_sb = []

            def emit_pb2():
                for oc in range(2):
                    pv = psum_s.tile([128, 1], F32, tag="ps", name="ps")
                    for p2 in range(2):
                        nc.tensor.matmul(pv[:],
                                         projT2p[:, p2, oc * 128:(oc + 1) * 128],
                                         bv_p2[p2][:], start=(p2 == 0), stop=(p2 == 1))
                    pb2 = gn.tile([128, 1], F32, tag=f"pb2_{oc}", name=f"pb2_{oc}")
                    nc.vector.tensor_add(pb2[:], pb_sb[oc][:], pv[:])
                    pb2_sb.append(pb2)

            # ------------- conv units (fp8 DoubleRow, woven in) ----------
            def emit_k(p, t8):
                kv = psum_s.tile([128, 512], F32, tag="ps", name="ps")
                nc.tensor.matmul(
                    kv[:], ws8[:, :, C + p * 128:C + (p + 1) * 128],
                    x8[:, :, t8 * 512:(t8 + 1) * 512],
                    start=True, stop=True, perf_mode=DR)
                # bias add on ScalarE: Lrelu(alpha=1) == identity, in-table
                nc.scalar.activation(
                    k_sb[p][:, t8 * 512:(t8 + 1) * 512], kv[:],
                    Ident, bias=b2_sb[2 + p][:])

            def emit_q(p, t4):
                qp = psum_s.tile([128, 512], F32, tag="ps", name="ps")
                nc.tensor.matmul(
                    qp[:], ws8[:, :, p * 128:(p + 1) * 128],
                    x8[:, :, t4 * 512:(t4 + 1) * 512],
                    start=True, stop=True, perf_mode=DR)
                nc.scalar.activation(
                    q_sb[p][:, t4 * 512:(t4 + 1) * 512], qp[:],
                    Ident, bias=b2_sb[p][:])

            def emit_vt(i, eng):
                vp = psum_s.tile([128, C], F32, tag="ps", name="ps")
                nc.tensor.matmul(
                    vp[:], x8[:, :, i * 128:(i + 1) * 128],
                    ws8[:, :, 2 * C:3 * C],
                    start=True, stop=True, perf_mode=DR)
                dst = vt5[:, i // 2, :, i % 2, 0:64]
                if eng == "s":
                    nc.scalar.copy(dst, vp[:].rearrange("p (h c) -> p h c", c=64))
                else:
                    nc.vector.tensor_copy(dst, vp[:].rearrange("p (h c) -> p h c", c=64))

            # per-(block, chunk) pre-emit schedule of conv units
            sched = {}

            def add_sched(b, i, fn):
                sched.setdefault((b, i), []).append(fn)

            add_sched(0, 0, lambda: emit_vt(0, "s"))
            add_sched(0, 0, lambda: emit_vt(1, "v"))
            add_sched(0, 0, lambda: emit_vt(2, "s"))
            for j in range(3, 32):
                b0c = (j - 3) // 2 + 1          # chunks 1..15, two vts per chunk
                add_sched(0, b0c, (lambda jj: lambda: emit_vt(
                    jj, "s" if jj % 2 else "v"))(j))
            for t8 in range(1, 8):
                add_sched(0, t8, (lambda t: lambda: emit_k(0, t))(t8))
            for t4 in range(1, 4):
                add_sched(t4 - 1, 20, (lambda t: lambda: emit_q(0, t))(t4))
            for t8 in range(8):
                add_sched(1 + t8 // 3, 8 + 3 * (t8 % 3), (lambda t: lambda: emit_k(1, t))(t8))
            for t4 in range(4):
                add_sched(2, 17 + 3 * t4, (lambda t: lambda: emit_q(1, t))(t4))
            add_sched(0, 16, emit_pb2)

            # ---------------- attention ----------------
            # 128-step pipeline (8 blocks x 16 key steps of 256). Per step:
            # two S sub-chunks (128 keys each) + their exps (fp8e5 out), with
            # the lagged PV DoubleRow matmuls interleaved between them so
            # every LDWEIGHTS hides behind the previous stream. Block's PV
            # steps 0+1 are emitted together at the step-1 slot so the a_ps
            # handoff never stalls on the previous block's drain.
            PV_LAG = 7
            ah_sb = {}
            post = {}          # g -> list of closures, run before step g

            def at(g, fn):
                post.setdefault(g, []).append(fn)

            def emit_pv_u(pblk, s2, u, p_t):
                p = pblk // NTT
                h = 2 * p + u
                rhs = p_t[:, u * 1024:(u + 1) * 1024].rearrange(
                    "p (j n) -> p j n", j=2)
                nc.tensor.matmul(
                    a_cur[pblk][:, u * 512:(u + 1) * 512],
                    vt5[:, s2, h, :, 0:65], rhs,
                    start=(s2 == 0), stop=(s2 == NS2 - 1), perf_mode=DR)

            def emit_drain(blk, g):
                p, tt = blk // NTT, blk % NTT
                a_ps = a_cur.pop(blk)
                araw = dn.tile([65, 1024], BF16, tag="araw", name="araw")
                last = (blk == 2 * NTT - 1)
                nc.scalar.copy(araw[:, 0:512], a_ps[:, 0:512])
                if last:
                    nc.vector.tensor_copy(araw[:, 512:1024], a_ps[:, 512:1024])
                else:
                    at(g + 1, lambda: nc.vector.tensor_copy(
                        araw[:, 512:1024], a_ps[:, 512:1024]))

                if last:
                    # pipelined per-half 1/Z off the raw psum accumulator:
                    # scalar/gpsimd/vector FIFOs overlap the halves
                    def fin_last():
                        zl = dn.tile([1, 1024], F32, tag="zl", name="zl")
                        zi = dn.tile([1, 1024], F32, tag="zi", name="zi")
                        d_bc = dn.tile([64, 1024], F32, tag="dbc", name="dbc")
                        ah = ahpool.tile([128, 512], BF16, tag=f"ah{p}_{tt}",
                                         name=f"ah{p}_{tt}")
                        for u in range(2):
                            hs = slice(u * 512, (u + 1) * 512)
                            nc.scalar.activation(zl[:, hs], a_ps[64:65, hs], Ln)
                            nc.scalar.activation(zi[:, hs], zl[:, hs], Exp,
                                                 scale=-1.0)
                            nc.gpsimd.partition_broadcast(d_bc[:, hs], zi[:, hs])
                            nc.vector.tensor_mul(ah[u * 64:(u + 1) * 64, :],
                                                 araw[0:64, hs], d_bc[:, hs])
                        ah_sb[(p, tt)] = ah
                    at(g + 1, fin_last)
                    at(g + 2, make_proj(tt))
                    return

                def ln():
                    zl = dn.tile([1, 1024], F32, tag="zl", name="zl")
                    nc.scalar.activation(zl[:], araw[64:65, :], Ln)

                    def inv_bc():
                        zi = dn.tile([1, 1024], F32, tag="zi", name="zi")
                        nc.scalar.activation(zi[:], zl[:], Exp, scale=-1.0)
                        d_bc = dn.tile([64, 1024], F32, tag="dbc", name="dbc")
                        nc.gpsimd.partition_broadcast(d_bc[:], zi[:])

                        def ah_fin():
                            ah = ahpool.tile([128, 512], BF16, tag=f"ah{p}_{tt}",
                                             name=f"ah{p}_{tt}")
                            for u in range(2):
                                nc.gpsimd.tensor_mul(
                                    ah[u * 64:(u + 1) * 64, :],
                                    araw[0:64, u * 512:(u + 1) * 512],
                                    d_bc[:, u * 512:(u + 1) * 512])
                            ah_sb[(p, tt)] = ah
                        at(g + 7, ah_fin)
                    at(g + 4, inv_bc)
                at(g + 2, ln)
                if p == 1:
                    at(g + 9, make_proj(tt))

            def make_proj(tt):
                def proj():
                    for oc in range(2):
                        pr = psum_s.tile([128, 512], F32, tag="ps", name="ps")
                        for p2 in range(2):
                            nc.tensor.matmul(
                                pr[:], projT2p[:, p2, oc * 128:(oc + 1) * 128],
                                ah_sb[(p2, tt)][:],
                                start=(p2 == 0), stop=(p2 == 1))
                        o1 = ao.tile([128, 512], F32, tag="o1", name="o1")
                        nc.vector.tensor_scalar_add(out=o1[:], in0=pr[:],
                                                    scalar1=pb2_sb[oc][:])
                        o2 = ao.tile([128, 512], F32, tag="o2", name="o2")
                        o2eng = nc.vector if tt == NTT - 1 else nc.gpsimd
                        o2eng.tensor_add(o2[:], o1[:],
                                         xb_sb[oc][:, tt * 512:(tt + 1) * 512])
                        nc.sync.dma_start(
                            out_d.ap()[oc * 128:(oc + 1) * 128,
                                       tt * 512:(tt + 1) * 512], o2[:])
                return proj

            emit_k(0, 0)
            emit_q(0, 0)

            NG = 2 * NTT * NS2          # 128 global steps
            pts = {}
            a_cur = {}

            def pv_units_for(g):
                gp = g - PV_LAG
                if gp < 0:
                    return []
                pblk, ps2 = gp // NS2, gp % NS2
                if ps2 == 0:
                    return []
                units = []
                if ps2 == 1:
                    units += [(pblk, 0, 0), (pblk, 0, 1)]
                units += [(pblk, ps2, 0), (pblk, ps2, 1)]
                return units

            def run_pv_units(units):
                for pblk, ps2, u in units:
                    if pblk not in a_cur:
                        a_cur[pblk] = psum_a.tile([65, 1024], F32,
                                                  tag="pa", name="pa")
                    emit_pv_u(pblk, ps2, u, pts[(pblk, ps2)])
                    if u == 1:
                        pts.pop((pblk, ps2))

            for g in range(NG + PV_LAG + 1):
                for fn in post.pop(g, ()):
                    fn()
                units = pv_units_for(g)
                if g < NG:
                    blk, s2 = g // NS2, g % NS2
                    p, tt = blk // NTT, blk % NTT
                    p_t = ppool.tile([128, 2048], FP8E5, tag="pt", name="pt")
                    p_t4 = p_t[:].rearrange("p (u j n) -> p u j n", u=2, j=2)
                    pts[(blk, s2)] = p_t
                    for j in range(2):
                        i = 2 * s2 + j
                        for fn in sched.pop((blk, i), ()):
                            fn()
                        s_ps = psum_s.tile([128, 1024], F32, tag="ps", name="ps")
                        for u in range(2):
                            nc.tensor.matmul(
                                s_ps[:, u * 512:(u + 1) * 512],
                                k_sb[p][u * 64:(u + 1) * 64, i * 128:(i + 1) * 128],
                                q_sb[p][u * 64:(u + 1) * 64, tt * 512:(tt + 1) * 512],
                                start=True, stop=True,
                                tile_position=(u * 64, 0))
                        dst = p_t4[:, :, j, :]
                        src = s_ps[:].rearrange("p (u n) -> p u n", u=2)
                        if j == 0:
                            nc.scalar.activation(dst, src, Exp,
                                                 scale=float(SCALE))
                        else:
                            nc.vector.tensor_scalar(
                                out=dst.bitcast(U8), in0=src,
                                scalar1=A5, scalar2=B5,
                                op0=mybir.AluOpType.mult,
                                op1=mybir.AluOpType.add)
                    run_pv_units(units)
                else:
                    run_pv_units(units)
                gp = g - PV_LAG
                if gp >= 0 and gp % NS2 == NS2 - 1:
                    emit_drain(gp // NS2, g)
            for g in range(NG + PV_LAG + 1, NG + PV_LAG + 12):
                for fn in post.pop(g, ()):
                    fn()
            assert not post and not sched and not pts

    nc.compile()
    nc.m = get_hw_module(nc.m)
    return nc


def _host_prep(inputs):
    x = np.asarray(inputs["x"], np.float32)
    gn_w = np.asarray(inputs["gn_weight"], np.float32)
    gn_b = np.asarray(inputs["gn_bias"], np.float32)
    qkv_w = np.asarray(inputs["qkv_w"], np.float32)
    qkv_b = np.asarray(inputs["qkv_b"], np.float32)
    proj_w = np.asarray(inputs["proj_w"], np.float32)
    proj_b = np.asarray(inputs["proj_b"], np.float32)

    W_ = qkv_w * gn_w[None, :]
    b_ = qkv_w @ gn_b + qkv_b
    qkvT = np.ascontiguousarray(W_.T).astype(BF)
    projT = np.ascontiguousarray(proj_w.T.reshape(HEADS, D, C)).astype(BF)

    gmat = np.zeros((128, 16), np.float32)
    gmatT = np.zeros((16, 128), np.float32)
    for ch in range(128):
        gmat[ch, ch // GS] = 1.0 / GS
        gmatT[ch // GS, ch] = 1.0
    shared = {
        "qkvT": qkvT,
        "qkvb": b_.reshape(3 * C, 1).astype(np.float32),
        "projT": projT,
        "projb": proj_b.reshape(C, 1).astype(np.float32),
        "gmat": gmat,
        "gmatT": gmatT,
    }
    x3 = x.reshape(B, C, T).astype(BF)
    in_maps = []
    for j in range(8):
        b, hf = j // 2, j % 2
        m = dict(shared)
        if hf == 0:
            m["xb"] = np.ascontiguousarray(x3[b])
        else:
            m["xb"] = np.ascontiguousarray(
                np.concatenate([x3[b][:, TQ:], x3[b][:, :TQ]], axis=1))
        m["x8"] = np.ascontiguousarray(
            m["xb"].reshape(2, 128, T).transpose(1, 0, 2)).astype(
                ml_dtypes.float8_e4m3fn)
        in_maps.append(m)
    return x, in_maps


def kernel(**inputs) -> np.ndarray:
    if "nc" not in _CACHED:
        _CACHED["nc"] = _build()
    nc = _CACHED["nc"]
    x, in_maps = _host_prep(inputs)
    res = bass_utils.run_bass_kernel_spmd(nc, in_maps, core_ids=list(range(8)))
    out = np.zeros((B, C, T), np.float32)
    for j in range(8):
        b, hf = j // 2, j % 2
        out[b][:, hf * TQ:(hf + 1) * TQ] = np.asarray(
            res.results[j]["out"], np.float32)
    return out.reshape(B, C, Himg, Wimg)


# revision 29
# speedup vs baseline: 1.0406x; 1.0027x over previous
"""AttentionBlock (GroupNorm -> qkv conv1x1 -> 4-head attention -> proj -> residual)
as a distributed Bass/Tile kernel on 8 TRN2 NeuronCores.

Sharding: core j handles batch b = j//2 and query-half h = j%2. The host
permutes x's spatial columns per core so queries are always cols 0:2048
(attention is permutation-invariant over keys). K/V are computed full-length
per core, so output slices are disjoint and no collectives are needed.

PV and the qkv convs run in fp8 DoubleRow (contraction = 2x128 per pass at
0.5 cyc/col): V is stored e4m3 with the per-head k-pair layout [128, 2, 80]
(j-stride %16==0 per the s3_lw dual-fp8 ISA rule), probabilities are e5m2,
conv weights/inputs e4m3 (x8 derived on device from the bf16 x). Softmax exp
is split between ScalarE (real Exp, fp8e5 out) and VectorE (Schraudolph
bit-trick: uint8 round of a*s+b ~= e5m2 bits of exp(s*scale); e5m2 chosen so
a +-6 sigma logit can't reach the NaN encoding). Z rides as a ones-row in
the DoubleRow weights; 1/Z via scalar Ln->Exp(-x), staggered over the
following steps. GroupNorm rstd via exp(-0.5*ln(var+eps)). Residual adds and
broadcasts run on GpSimd. Conv weights are rstd-folded on device so PE conv
work starts without waiting for GN statistics to be applied to x. The first
two PV steps of each block are emitted together one step late so the psum
accumulator handoff never stalls on the previous block's drain.
"""
import numpy as np
import ml_dtypes

import concourse.bass as bass
import concourse.bacc as bacc
import concourse.tile as tile
from concourse import mybir
from concourse import bass_utils
from concourse.bass_interp import get_hw_module

F32 = mybir.dt.float32
BF16 = mybir.dt.bfloat16
FP8E4 = mybir.dt.float8e4
FP8E5 = mybir.dt.float8e5
U8 = mybir.dt.uint8
BF = ml_dtypes.bfloat16

B, C, Himg, Wimg = 4, 256, 64, 64
T = Himg * Wimg            # 4096 tokens
HEADS, D = 4, 64           # 4 heads x 64 dims
GROUPS, GS = 32, 8         # groupnorm: 32 groups of 8 channels
EPS = 1e-5
TQ = T // 2                # queries per core (2048)
NTT = TQ // 512            # query tiles of 512
NSC = T // 128             # 128-key chunks (32)
NS2 = T // 256             # 256-key PV steps (16)
SCALE = 1.0 / np.sqrt(D)
LOG2E = float(np.log2(np.e))
A5 = float(4.0 * LOG2E * SCALE)     # schraudolph mult (e5m2 bits)
B5 = float(60.0 - 0.24)             # schraudolph bias (weighted-rms centering)
Exp = mybir.ActivationFunctionType.Exp
Ln = mybir.ActivationFunctionType.Ln
Ident = mybir.ActivationFunctionType.Identity
DR = mybir.MatmulPerfMode.DoubleRow

_CACHED = {}


def _patch_act_tables():
    """Restrict the act-table chooser to natural_log_exp_and_others so the
    scalar engine never reloads tables (exp+ln live in one set; identity
    copies are expressed as Identity, also in that set). Set order is
    preserved so act_func_set_id stays aligned with act_info.json."""
    if getattr(bacc, "_act_tables_patched", False):
        return
    orig = bacc.get_activation_tables

    def patched(arch):
        t = orig(arch)
        return {name: (fns if name == "natural_log_exp_and_others" else set())
                for name, fns in t.items()}

    bacc.get_activation_tables = patched
    bacc._act_tables_patched = True


def _build():
    _patch_act_tables()
    nc = bacc.Bacc("TRN2", target_bir_lowering=False, debug=False,
                   enable_asserts=False, num_devices=8)

    xb_d = nc.dram_tensor("xb", [C, T], BF16, kind="ExternalInput")
    x8_d = nc.dram_tensor("x8", [128, 2, T], FP8E4, kind="ExternalInput")
    qkvT_d = nc.dram_tensor("qkvT", [C, 3 * C], BF16, kind="ExternalInput")
    qkvb_d = nc.dram_tensor("qkvb", [3 * C, 1], F32, kind="ExternalInput")
    projT_d = nc.dram_tensor("projT", [HEADS, D, C], BF16, kind="ExternalInput")
    projb_d = nc.dram_tensor("projb", [C, 1], F32, kind="ExternalInput")
    gmat_d = nc.dram_tensor("gmat", [128, 16], F32, kind="ExternalInput")
    gmatT_d = nc.dram_tensor("gmatT", [16, 128], F32, kind="ExternalInput")
    out_d = nc.dram_tensor("out", [C, TQ], F32, kind="ExternalOutput")

    with tile.TileContext(nc) as tc:
        with (
            tc.tile_pool(name="consts", bufs=1) as consts,
            tc.tile_pool(name="data", bufs=1) as data,
            tc.tile_pool(name="gn", bufs=1) as gn,
            tc.tile_pool(name="pt", bufs=10) as ppool,
            tc.tile_pool(name="dn", bufs=2) as dn,
            tc.tile_pool(name="ao", bufs=4) as ao,
            tc.tile_pool(name="ah", bufs=1) as ahpool,
            tc.tile_pool(name="ps", bufs=3, space="PSUM") as psum_s,
            tc.tile_pool(name="pa", bufs=1, space="PSUM") as psum_a,
        ):
            # ---------------- tiles ----------------
            qkvT2 = consts.tile([128, 2, 3 * C], BF16, tag="qkvT2", name="qkvT2")
            qkvT_sb = [qkvT2[:, ct, :] for ct in range(2)]
            projT2p = consts.tile([128, 2, C], BF16, tag="projT2p", name="projT2p")
            b8 = consts.tile([128, 8], F32, tag="b8", name="b8")
            b_in = [b8[:, m:m + 1] for m in range(6)]
            pb_sb = [b8[:, 6 + oc:7 + oc] for oc in range(2)]
            gmat_sb = consts.tile([128, 16], F32, tag="gmat", name="gmat")
            gmatT_sb = consts.tile([16, 128], F32, tag="gmatT", name="gmatT")
            eps_t = gn.tile([16, 1], F32, tag="eps", name="eps")
            nc.vector.memset(eps_t[:], EPS)
            # preload the exp/ln act table while DMAs are in flight
            warm = gn.tile([16, 1], F32, tag="warm", name="warm")
            nc.scalar.activation(warm[:], eps_t[:], Exp)

            xb_sb, st_sb = [], []
            for ct in range(2):
                xt = data.tile([128, T], BF16, tag=f"xb{ct}", name=f"xb{ct}")
                xb_sb.append(xt)
                st = gn.tile([128, 8, 6], F32, tag=f"st{ct}", name=f"st{ct}")
                st_sb.append(st)
            # fp8 copy of x in conv DoubleRow pair layout (ct pairs on the
            # free dim), host-converted
            x8 = data.tile([128, 2, T], FP8E4, tag="x8", name="x8")

            # ---------------- loads: x chunks first, weights woven -------
            qdma = [nc.sync, nc.scalar, nc.gpsimd]
            for c8 in range(8):
                for ct in range(2):
                    sl = slice(c8 * 512, (c8 + 1) * 512)
                    qdma[(2 * c8 + ct) % 3].dma_start(
                        xb_sb[ct][:, sl], xb_d.ap()[ct * 128:(ct + 1) * 128, sl])
            nc.scalar.dma_start(qkvT2[:], qkvT_d.ap()[:].rearrange(
                "(ct p) o -> p ct o", ct=2))
            nc.sync.dma_start(b8[:, 0:6], qkvb_d.ap()[:].rearrange(
                "(m p) one -> p (m one)", m=6))
            nc.sync.dma_start(b8[:, 6:8], projb_d.ap()[:].rearrange(
                "(oc p) one -> p (oc one)", oc=2))
            nc.sync.dma_start(projT2p[:], projT_d.ap()[:].rearrange(
                "(p2 u) d o -> (u d) p2 o", p2=2))
            nc.gpsimd.dma_start(gmat_sb[:], gmat_d.ap()[:])
            nc.gpsimd.dma_start(gmatT_sb[:], gmatT_d.ap()[:])
            # fp8 x behind the bf16 x (needed only once convs start)
            for c8 in range(8):
                sl = slice(c8 * 512, (c8 + 1) * 512)
                qdma[c8 % 3].dma_start(x8[:, :, sl], x8_d.ap()[:, :, sl])
            # PE warm-up: junk matmuls on the first landed chunk keep the
            # HAM activity monitor busy so the real pipeline starts at
            # 2.4GHz instead of cold 1.2GHz.
            junk_ps = psum_s.tile([128, 512], F32, tag="ps", name="ps")
            for _ in range(32):
                nc.tensor.matmul(junk_ps[:], xb_sb[0][0:128, 0:128],
                                 xb_sb[0][:, 0:512], start=True, stop=True)
            for c4 in range(4):
                for ct in range(2):
                    for half in range(2):
                        sh = slice(c4 * 1024 + half * 512,
                                   c4 * 1024 + (half + 1) * 512)
                        nc.vector.bn_stats(st_sb[ct][:, 2 * c4 + half, :],
                                           xb_sb[ct][:, sh])
            # ---------------- GroupNorm statistics ----------------
            stats2 = []
            for ct in range(2):
                mv = gn.tile([128, 2], F32, tag=f"mv{ct}", name=f"mv{ct}")
                nc.vector.bn_aggr(mv[:], st_sb[ct][:])
                s2 = gn.tile([128, 2], F32, tag=f"s2{ct}", name=f"s2{ct}")
                nc.vector.tensor_copy(s2[:, 0:1], mv[:, 0:1])
                m2 = gn.tile([128, 1], F32, tag=f"m2{ct}", name=f"m2{ct}")
                nc.vector.tensor_mul(m2[:], mv[:, 0:1], mv[:, 0:1])
                nc.vector.tensor_add(s2[:, 1:2], m2[:], mv[:, 1:2])
                stats2.append(s2)

            # group (mean, E[x^2]) -> per-group rstd via exp(-0.5*ln(var+eps))
            gs_ps, bc_sb = [], []
            vg = gn.tile([16, 2], F32, tag="vg", name="vg")
            for ct in range(2):
                g1 = psum_s.tile([16, 2], F32, tag="ps", name="ps")
                nc.tensor.matmul(g1[:], gmat_sb[:], stats2[ct][:],
                                 start=True, stop=True)
                gsb = gn.tile([16, 2], F32, tag=f"gsb{ct}", name=f"gsb{ct}")
                nc.vector.tensor_copy(gsb[:], g1[:])
                gs_ps.append(gsb)
                m2g = gn.tile([16, 1], F32, tag=f"m2g{ct}", name=f"m2g{ct}")
                nc.vector.tensor_mul(m2g[:], gsb[:, 0:1], gsb[:, 0:1])
                nc.vector.tensor_sub(vg[:, ct:ct + 1], gsb[:, 1:2], m2g[:])
            for _ in range(6):
                nc.tensor.matmul(junk_ps[:], xb_sb[0][0:128, 0:128],
                                 xb_sb[0][:, 0:512], start=True, stop=True)
            lgv = gn.tile([16, 2], F32, tag="lgv", name="lgv")
            nc.scalar.activation(lgv[:], vg[:], Ln, bias=eps_t[:])
            rg = gn.tile([16, 2], F32, tag="rg", name="rg")
            nc.scalar.activation(rg[:], lgv[:], Exp, scale=-0.5)
            for ct in range(2):
                bcv = gn.tile([16, 2], F32, tag=f"bcv{ct}", name=f"bcv{ct}")
                nc.vector.tensor_copy(bcv[:, 0:1], gs_ps[ct][:, 0:1])
                nc.vector.tensor_copy(bcv[:, 1:2], rg[:, ct:ct + 1])
                b1 = psum_s.tile([128, 2], F32, tag="ps", name="ps")
                nc.tensor.matmul(b1[:], gmatT_sb[:], bcv[:],
                                 start=True, stop=True)
                bsb = gn.tile([128, 2], F32, tag=f"bc{ct}", name=f"bc{ct}")
                nc.vector.tensor_copy(bsb[:], b1[:])
                bc_sb.append(bsb)

            for _ in range(8):
                nc.tensor.matmul(junk_ps[:], xb_sb[0][0:128, 0:128],
                                 xb_sb[0][:, 0:512], start=True, stop=True)
            # ------- fold rstd into weights (bf16 for the mu path, fp8 for
            # the convs); bias b2 = b - W'mu -------
            qkvS_sb, mu_bf = [], []
            ws8 = consts.tile([128, 2, 3 * C], FP8E4, tag="ws8", name="ws8")
            for ct in range(2):
                ws = consts.tile([128, 3 * C], BF16, tag=f"qkvS{ct}", name=f"qkvS{ct}")
                nc.vector.tensor_scalar_mul(out=ws[:], in0=qkvT_sb[ct][:],
                                            scalar1=bc_sb[ct][:, 1:2])
                qkvS_sb.append(ws)
                nc.vector.tensor_scalar_mul(out=ws8[:, ct, :], in0=qkvT_sb[ct][:],
                                            scalar1=bc_sb[ct][:, 1:2])
                mb = gn.tile([128, 1], BF16, tag=f"mub{ct}", name=f"mub{ct}")
                nc.vector.tensor_copy(mb[:], bc_sb[ct][:, 0:1])
                mu_bf.append(mb)
            b2_sb = [None] * 6
            for m in (2, 3, 0, 1, 4, 5):     # k biases first: K(0,0) gates S
                wm = psum_s.tile([128, 1], F32, tag="ps", name="ps")
                for ct in range(2):
                    nc.tensor.matmul(wm[:], qkvS_sb[ct][:, m * 128:(m + 1) * 128],
                                     mu_bf[ct][:], start=(ct == 0), stop=(ct == 1))
                b2 = gn.tile([128, 1], F32, tag=f"b2_{m}", name=f"b2_{m}")
                nc.vector.tensor_sub(b2[:], b_in[m][:], wm[:])
                b2_sb[m] = b2
            # bf16 copies of the v-slice biases for the proj-bias fold;
            # head pair (2*p2, 2*p2+1) sits at partitions 0-63 / 64-127.
            bv_p2 = []
            for m in (4, 5):
                bb = gn.tile([128, 1], BF16, tag=f"bv{m}", name=f"bv{m}")
                nc.vector.tensor_copy(bb[:], b2_sb[m][:])
                bv_p2.append(bb)

            # ---------------- SBUF destination tiles ----------------
            k_sb = [data.tile([128, T], BF16, tag=f"k{p}", name=f"k{p}")
                    for p in range(2)]
            q_sb = [data.tile([128, TQ], BF16, tag=f"q{p}", name=f"q{p}")
                    for p in range(2)]
            # V in fp8e4, PV-DoubleRow layout: (s2, h, j, c) with c-stride 80
            # (j-step %16==0 per the s3_lw dual-fp8 rule); c=64 is the ones
            # row that accumulates Z.
            vt_sb = data.tile([128, NS2 * HEADS * 2 * 80], FP8E4,
                              tag="vt", name="vt")
            vt5 = vt_sb[:].rearrange("p (s h j c) -> p s h j c",
                                     s=NS2, h=HEADS, j=2, c=80)
            nc.vector.memset(vt5[:, :, :, :, 64:65], 1.0)

            # pb2[oc] = projb[oc] + sum_h projT_h[:,oc]^T @ bv_h
            pb2_sb = []

            def emit_pb2():
                for oc in range(2):
                    pv = psum_s.tile([128, 1], F32, tag="ps", name="ps")
                    for p2 in range(2):
                        nc.tensor.matmul(pv[:],
                                         projT2p[:, p2, oc * 128:(oc + 1) * 128],
                                         bv_p2[p2][:], start=(p2 == 0), stop=(p2 == 1))
                    pb2 = gn.tile([128, 1], F32, tag=f"pb2_{oc}", name=f"pb2_{oc}")
                    nc.vector.tensor_add(pb2[:], pb_sb[oc][:], pv[:])
                    pb2_sb.append(pb2)

            # ------------- conv units (fp8 DoubleRow, woven in) ----------
            def emit_k(p, t8):
                kv = psum_s.tile([128, 512], F32, tag="ps", name="ps")
                nc.tensor.matmul(
                    kv[:], ws8[:, :, C + p * 128:C + (p + 1) * 128],
                    x8[:, :, t8 * 512:(t8 + 1) * 512],
                    start=True, stop=True, perf_mode=DR)
                # bias add on ScalarE: Lrelu(alpha=1) == identity, in-table
                nc.scalar.activation(
                    k_sb[p][:, t8 * 512:(t8 + 1) * 512], kv[:],
                    Ident, bias=b2_sb[2 + p][:])

            def emit_q(p, t4):
                qp = psum_s.tile([128, 512], F32, tag="ps", name="ps")
                nc.tensor.matmul(
                    qp[:], ws8[:, :, p * 128:(p + 1) * 128],
                    x8[:, :, t4 * 512:(t4 + 1) * 512],
                    start=True, stop=True, perf_mode=DR)
                nc.scalar.activation(
                    q_sb[p][:, t4 * 512:(t4 + 1) * 512], qp[:],
                    Ident, bias=b2_sb[p][:])

            def emit_vt(i, eng):
                vp = psum_s.tile([128, C], F32, tag="ps", name="ps")
                nc.tensor.matmul(
                    vp[:], x8[:, :, i * 128:(i + 1) * 128],
                    ws8[:, :, 2 * C:3 * C],
                    start=True, stop=True, perf_mode=DR)
                dst = vt5[:, i // 2, :, i % 2, 0:64]
                if eng == "s":
                    nc.scalar.copy(dst, vp[:].rearrange("p (h c) -> p h c", c=64))
                else:
                    nc.vector.tensor_copy(dst, vp[:].rearrange("p (h c) -> p h c", c=64))

            # per-(block, chunk) pre-emit schedule of conv units
            sched = {}

            def add_sched(b, i, fn):
                sched.setdefault((b, i), []).append(fn)

            add_sched(0, 0, lambda: emit_vt(0, "s"))
            add_sched(0, 0, lambda: emit_vt(1, "v"))
            add_sched(0, 0, lambda: emit_vt(2, "s"))
            for j in range(3, 32):
                b0c = (j - 3) // 2 + 1          # chunks 1..15, two vts per chunk
                add_sched(0, b0c, (lambda jj: lambda: emit_vt(
                    jj, "s" if jj % 2 else "v"))(j))
            for t8 in range(1, 8):
                add_sched(0, t8, (lambda t: lambda: emit_k(0, t))(t8))
            for t4 in range(1, 4):
                add_sched(t4 - 1, 20, (lambda t: lambda: emit_q(0, t))(t4))
            for t8 in range(8):
                add_sched(1 + t8 // 3, 8 + 3 * (t8 % 3), (lambda t: lambda: emit_k(1, t))(t8))
            for t4 in range(4):
                add_sched(2, 17 + 3 * t4, (lambda t: lambda: emit_q(1, t))(t4))
            add_sched(0, 16, emit_pb2)

            # ---------------- attention ----------------
            # 128-step pipeline (8 blocks x 16 key steps of 256). Per step:
            # two S sub-chunks (128 keys each) + their exps (fp8e5 out), with
            # the lagged PV DoubleRow matmuls interleaved between them so
            # every LDWEIGHTS hides behind the previous stream. Block's PV
            # steps 0+1 are emitted together at the step-1 slot so the a_ps
            # handoff never stalls on the previous block's drain.
            PV_LAG = 7
            ah_sb = {}
            post = {}          # g -> list of closures, run before step g

            def at(g, fn):
                post.setdefault(g, []).append(fn)

            def emit_pv_u(pblk, s2, u, p_t):
                p = pblk // NTT
                h = 2 * p + u
                rhs = p_t[:, u * 1024:(u + 1) * 1024].rearrange(
                    "p (j n) -> p j n", j=2)
                nc.tensor.matmul(
                    a_cur[pblk][:, u * 512:(u + 1) * 512],
                    vt5[:, s2, h, :, 0:65], rhs,
                    start=(s2 == 0), stop=(s2 == NS2 - 1), perf_mode=DR)

            def emit_drain(blk, g):
                p, tt = blk // NTT, blk % NTT
                a_ps = a_cur.pop(blk)
                araw = dn.tile([65, 1024], BF16, tag="araw", name="araw")
                last = (blk == 2 * NTT - 1)
                nc.scalar.copy(araw[:, 0:512], a_ps[:, 0:512])
                if last:
                    nc.vector.tensor_copy(araw[:, 512:1024], a_ps[:, 512:1024])
                else:
                    at(g + 1, lambda: nc.vector.tensor_copy(
                        araw[:, 512:1024], a_ps[:, 512:1024]))

                if last:
                    # pipelined per-half 1/Z off the raw psum accumulator:
                    # scalar/gpsimd/vector FIFOs overlap the halves
                    def fin_last():
                        zl = dn.tile([1, 1024], F32, tag="zl", name="zl")
                        zi = dn.tile([1, 1024], F32, tag="zi", name="zi")
                        d_bc = dn.tile([64, 1024], F32, tag="dbc", name="dbc")
                        ah = ahpool.tile([128, 512], BF16, tag=f"ah{p}_{tt}",
                                         name=f"ah{p}_{tt}")
                        for u in range(2):
                            hs = slice(u * 512, (u + 1) * 512)
                            nc.scalar.activation(zl[:, hs], a_ps[64:65, hs], Ln)
                            nc.scalar.activation(zi[:, hs], zl[:, hs], Exp,
                                                 scale=-1.0)
                            nc.gpsimd.partition_broadcast(d_bc[:, hs], zi[:, hs])
                            nc.vector.tensor_mul(ah[u * 64:(u + 1) * 64, :],
                                                 araw[0:64, hs], d_bc[:, hs])
                        ah_sb[(p, tt)] = ah
                    at(g + 1, fin_last)
                    at(g + 2, make_proj(tt))
                    return

                def ln():
                    zl = dn.tile([1, 1024], F32, tag="zl", name="zl")
                    nc.scalar.activation(zl[:], araw[64:65, :], Ln)

                    def inv_bc():
                        zi = dn.tile([1, 1024], F32, tag="zi", name="zi")
                        nc.scalar.activation(zi[:], zl[:], Exp, scale=-1.0)
                        d_bc = dn.tile([64, 1024], F32, tag="dbc", name="dbc")
                        nc.gpsimd.partition_broadcast(d_bc[:], zi[:])

                        def ah_fin():
                            ah = ahpool.tile([128, 512], BF16, tag=f"ah{p}_{tt}",
                                             name=f"ah{p}_{tt}")
                            for u in range(2):
                                nc.gpsimd.tensor_mul(
                                    ah[u * 64:(u + 1) * 64, :],
                                    araw[0:64, u * 512:(u + 1) * 512],
                                    d_bc[:, u * 512:(u + 1) * 512])
                            ah_sb[(p, tt)] = ah
                        at(g + 7, ah_fin)
                    at(g + 4, inv_bc)
                at(g + 2, ln)
                if p == 1:
                    at(g + 9, make_proj(tt))

            def make_proj(tt):
                def proj():
                    for oc in range(2):
                        pr = psum_s.tile([128, 512], F32, tag="ps", name="ps")
                        for p2 in range(2):
                            nc.tensor.matmul(
                                pr[:], projT2p[:, p2, oc * 128:(oc + 1) * 128],
                                ah_sb[(p2, tt)][:],
                                start=(p2 == 0), stop=(p2 == 1))
                        o1 = ao.tile([128, 512], F32, tag="o1", name="o1")
                        nc.vector.tensor_scalar_add(out=o1[:], in0=pr[:],
                                                    scalar1=pb2_sb[oc][:])
                        o2 = ao.tile([128, 512], F32, tag="o2", name="o2")
                        o2eng = nc.vector if tt == NTT - 1 else nc.gpsimd
                        o2eng.tensor_add(o2[:], o1[:],
                                         xb_sb[oc][:, tt * 512:(tt + 1) * 512])
                        nc.sync.dma_start(
                            out_d.ap()[oc * 128:(oc + 1) * 128,
                                       tt * 512:(tt + 1) * 512], o2[:])
                return proj

            emit_k(0, 0)
            emit_q(0, 0)

            NG = 2 * NTT * NS2          # 128 global steps
            pts = {}
            a_cur = {}

            def pv_units_for(g):
                gp = g - PV_LAG
                if gp < 0:
                    return []
                pblk, ps2 = gp // NS2, gp % NS2
                if ps2 == 0:
                    return []
                units = []
                if ps2 == 1:
                    units += [(pblk, 0, 0), (pblk, 0, 1)]
                units += [(pblk, ps2, 0), (pblk, ps2, 1)]
                return units

            def run_pv_units(units):
                for pblk, ps2, u in units:
                    if pblk not in a_cur:
                        a_cur[pblk] = psum_a.tile([65, 1024], F32,
                                                  tag="pa", name="pa")
                    emit_pv_u(pblk, ps2, u, pts[(pblk, ps2)])
                    if u == 1:
                        pts.pop((pblk, ps2))

            for g in range(NG + PV_LAG + 1):
                for fn in post.pop(g, ()):
                    fn()
                units = pv_units_for(g)
                if g < NG:
                    run_pv_units(units)
                    blk, s2 = g // NS2, g % NS2
                    p, tt = blk // NTT, blk % NTT
                    p_t = ppool.tile([128, 2048], FP8E5, tag="pt", name="pt")
                    p_t4 = p_t[:].rearrange("p (u j n) -> p u j n", u=2, j=2)
                    pts[(blk, s2)] = p_t
                    for j in range(2):
                        i = 2 * s2 + j
                        for fn in sched.pop((blk, i), ()):
                            fn()
                        s_ps = psum_s.tile([128, 1024], F32, tag="ps", name="ps")
                        for u in range(2):
                            nc.tensor.matmul(
                                s_ps[:, u * 512:(u + 1) * 512],
                                k_sb[p][u * 64:(u + 1) * 64, i * 128:(i + 1) * 128],
                                q_sb[p][u * 64:(u + 1) * 64, tt * 512:(tt + 1) * 512],
                                start=True, stop=True,
                                tile_position=(u * 64, 0))
                        dst = p_t4[:, :, j, :]
                        src = s_ps[:].rearrange("p (u n) -> p u n", u=2)
                        if j == 0:
                            nc.scalar.activation(dst, src, Exp,
                                                 scale=float(SCALE))
                        else:
                            nc.vector.tensor_scalar(
                                out=dst.bitcast(U8), in0=src,
                                scalar1=A5, scalar2=B5,
                                op0=mybir.AluOpType.mult,
                                op1=mybir.AluOpType.add)
                else:
                    run_pv_units(units)
                gp = g - PV_LAG
                if gp >= 0 and gp % NS2 == NS2 - 1:
                    emit_drain(gp // NS2, g)
            for g in range(NG + PV_LAG + 1, NG + PV_LAG + 12):
                for fn in post.pop(g, ()):
                    fn()
            assert not post and not sched and not pts

    nc.compile()
    nc.m = get_hw_module(nc.m)
    return nc


def _host_prep(inputs):
    x = np.asarray(inputs["x"], np.float32)
    gn_w = np.asarray(inputs["gn_weight"], np.float32)
    gn_b = np.asarray(inputs["gn_bias"], np.float32)
    qkv_w = np.asarray(inputs["qkv_w"], np.float32)
    qkv_b = np.asarray(inputs["qkv_b"], np.float32)
    proj_w = np.asarray(inputs["proj_w"], np.float32)
    proj_b = np.asarray(inputs["proj_b"], np.float32)

    W_ = qkv_w * gn_w[None, :]
    b_ = qkv_w @ gn_b + qkv_b
    qkvT = np.ascontiguousarray(W_.T).astype(BF)
    projT = np.ascontiguousarray(proj_w.T.reshape(HEADS, D, C)).astype(BF)

    gmat = np.zeros((128, 16), np.float32)
    gmatT = np.zeros((16, 128), np.float32)
    for ch in range(128):
        gmat[ch, ch // GS] = 1.0 / GS
        gmatT[ch // GS, ch] = 1.0
    shared = {
        "qkvT": qkvT,
        "qkvb": b_.reshape(3 * C, 1).astype(np.float32),
        "projT": projT,
        "projb": proj_b.reshape(C, 1).astype(np.float32),
        "gmat": gmat,
        "gmatT": gmatT,
    }
    x3 = x.reshape(B, C, T).astype(BF)
    in_maps = []
    for j in range(8):
        b, hf = j // 2, j % 2
        m = dict(shared)
        if hf == 0:
            m["xb"] = np.ascontiguousarray(x3[b])
        else:
            m["xb"] = np.ascontiguousarray(
                np.concatenate([x3[b][:, TQ:], x3[b][:, :TQ]], axis=1))
        m["x8"] = np.ascontiguousarray(
            m["xb"].reshape(2, 128, T).transpose(1, 0, 2)).astype(
                ml_dtypes.float8_e4m3fn)
        in_maps.append(m)
    return x, in_maps


def kernel(**inputs) -> np.ndarray:
    if "nc" not in _CACHED:
        _CACHED["nc"] = _build()
    nc = _CACHED["nc"]
    x, in_maps = _host_prep(inputs)
    res = bass_utils.run_bass_kernel_spmd(nc, in_maps, core_ids=list(range(8)))
    out = np.zeros((B, C, T), np.float32)
    for j in range(8):
        b, hf = j // 2, j % 2
        out[b][:, hf * TQ:(hf + 1) * TQ] = np.asarray(
            res.results[j]["out"], np.float32)
    return out.reshape(B, C, Himg, Wimg)


# revision 30
# speedup vs baseline: 1.0409x; 1.0003x over previous
"""AttentionBlock (GroupNorm -> qkv conv1x1 -> 4-head attention -> proj -> residual)
as a distributed Bass/Tile kernel on 8 TRN2 NeuronCores.

Sharding: core j handles batch b = j//2 and query-half h = j%2. The host
permutes x's spatial columns per core so queries are always cols 0:2048
(attention is permutation-invariant over keys). K/V are computed full-length
per core, so output slices are disjoint and no collectives are needed.

PV and the qkv convs run in fp8 DoubleRow (contraction = 2x128 per pass at
0.5 cyc/col): V is stored e4m3 with the per-head k-pair layout [128, 2, 80]
(j-stride %16==0 per the s3_lw dual-fp8 ISA rule), probabilities are e5m2,
conv weights/inputs e4m3 (x8 derived on device from the bf16 x). Softmax exp
is split between ScalarE (real Exp, fp8e5 out) and VectorE (Schraudolph
bit-trick: uint8 round of a*s+b ~= e5m2 bits of exp(s*scale); e5m2 chosen so
a +-6 sigma logit can't reach the NaN encoding). Z rides as a ones-row in
the DoubleRow weights; 1/Z via scalar Ln->Exp(-x), staggered over the
following steps. GroupNorm rstd via exp(-0.5*ln(var+eps)). Residual adds and
broadcasts run on GpSimd. Conv weights are rstd-folded on device so PE conv
work starts without waiting for GN statistics to be applied to x. The first
two PV steps of each block are emitted together one step late so the psum
accumulator handoff never stalls on the previous block's drain.
"""
import numpy as np
import ml_dtypes

import concourse.bass as bass
import concourse.bacc as bacc
import concourse.tile as tile
from concourse import mybir
from concourse import bass_utils
from concourse.bass_interp import get_hw_module

F32 = mybir.dt.float32
BF16 = mybir.dt.bfloat16
FP8E4 = mybir.dt.float8e4
FP8E5 = mybir.dt.float8e5
U8 = mybir.dt.uint8
BF = ml_dtypes.bfloat16

B, C, Himg, Wimg = 4, 256, 64, 64
T = Himg * Wimg            # 4096 tokens
HEADS, D = 4, 64           # 4 heads x 64 dims
GROUPS, GS = 32, 8         # groupnorm: 32 groups of 8 channels
EPS = 1e-5
TQ = T // 2                # queries per core (2048)
NTT = TQ // 512            # query tiles of 512
NSC = T // 128             # 128-key chunks (32)
NS2 = T // 256             # 256-key PV steps (16)
SCALE = 1.0 / np.sqrt(D)
LOG2E = float(np.log2(np.e))
A5 = float(4.0 * LOG2E * SCALE)     # schraudolph mult (e5m2 bits)
B5 = float(60.0 - 0.24)             # schraudolph bias (weighted-rms centering)
Exp = mybir.ActivationFunctionType.Exp
Ln = mybir.ActivationFunctionType.Ln
Ident = mybir.ActivationFunctionType.Identity
DR = mybir.MatmulPerfMode.DoubleRow

_CACHED = {}


def _patch_act_tables():
    """Restrict the act-table chooser to natural_log_exp_and_others so the
    scalar engine never reloads tables (exp+ln live in one set; identity
    copies are expressed as Identity, also in that set). Set order is
    preserved so act_func_set_id stays aligned with act_info.json."""
    if getattr(bacc, "_act_tables_patched", False):
        return
    orig = bacc.get_activation_tables

    def patched(arch):
        t = orig(arch)
        return {name: (fns if name == "natural_log_exp_and_others" else set())
                for name, fns in t.items()}

    bacc.get_activation_tables = patched
    bacc._act_tables_patched = True


def _build():
    _patch_act_tables()
    nc = bacc.Bacc("TRN2", target_bir_lowering=False, debug=False,
                   enable_asserts=False, num_devices=8)

    xb_d = nc.dram_tensor("xb", [C, TQ], BF16, kind="ExternalInput")
    x8_d = nc.dram_tensor("x8", [128, 2, T], FP8E4, kind="ExternalInput")
    qkvT_d = nc.dram_tensor("qkvT", [C, 3 * C], BF16, kind="ExternalInput")
    qkvb_d = nc.dram_tensor("qkvb", [3 * C, 1], F32, kind="ExternalInput")
    projT_d = nc.dram_tensor("projT", [HEADS, D, C], BF16, kind="ExternalInput")
    projb_d = nc.dram_tensor("projb", [C, 1], F32, kind="ExternalInput")
    gmat_d = nc.dram_tensor("gmat", [128, 16], F32, kind="ExternalInput")
    gmatT_d = nc.dram_tensor("gmatT", [16, 128], F32, kind="ExternalInput")
    out_d = nc.dram_tensor("out", [C, TQ], F32, kind="ExternalOutput")

    with tile.TileContext(nc) as tc:
        with (
            tc.tile_pool(name="consts", bufs=1) as consts,
            tc.tile_pool(name="data", bufs=1) as data,
            tc.tile_pool(name="gn", bufs=1) as gn,
            tc.tile_pool(name="pt", bufs=10) as ppool,
            tc.tile_pool(name="dn", bufs=2) as dn,
            tc.tile_pool(name="ao", bufs=4) as ao,
            tc.tile_pool(name="ah", bufs=1) as ahpool,
            tc.tile_pool(name="ps", bufs=3, space="PSUM") as psum_s,
            tc.tile_pool(name="pa", bufs=1, space="PSUM") as psum_a,
        ):
            # ---------------- tiles ----------------
            qkvT2 = consts.tile([128, 2, 3 * C], BF16, tag="qkvT2", name="qkvT2")
            qkvT_sb = [qkvT2[:, ct, :] for ct in range(2)]
            projT2p = consts.tile([128, 2, C], BF16, tag="projT2p", name="projT2p")
            b8 = consts.tile([128, 8], F32, tag="b8", name="b8")
            b_in = [b8[:, m:m + 1] for m in range(6)]
            pb_sb = [b8[:, 6 + oc:7 + oc] for oc in range(2)]
            gmat_sb = consts.tile([128, 16], F32, tag="gmat", name="gmat")
            gmatT_sb = consts.tile([16, 128], F32, tag="gmatT", name="gmatT")
            eps_t = gn.tile([16, 1], F32, tag="eps", name="eps")
            nc.vector.memset(eps_t[:], EPS)
            # preload the exp/ln act table while DMAs are in flight
            warm = gn.tile([16, 1], F32, tag="warm", name="warm")
            nc.scalar.activation(warm[:], eps_t[:], Exp)

            xb_sb, st_sb = [], []
            for ct in range(2):
                xt = data.tile([128, TQ], BF16, tag=f"xb{ct}", name=f"xb{ct}")
                xb_sb.append(xt)
                st = gn.tile([128, 8, 6], F32, tag=f"st{ct}", name=f"st{ct}")
                st_sb.append(st)
            # fp8 copy of x in conv DoubleRow pair layout (ct pairs on the
            # free dim), host-converted
            x8 = data.tile([128, 2, T], FP8E4, tag="x8", name="x8")

            # ---------------- loads: x chunks first, weights woven -------
            qdma = [nc.sync, nc.scalar, nc.gpsimd]
            nc.sync.dma_start(xb_sb[0][:, 0:512], xb_d.ap()[0:128, 0:512])
            for c8 in range(8):
                sl = slice(c8 * 512, (c8 + 1) * 512)
                qdma[c8 % 3].dma_start(x8[:, :, sl], x8_d.ap()[:, :, sl])
            nc.scalar.dma_start(qkvT2[:], qkvT_d.ap()[:].rearrange(
                "(ct p) o -> p ct o", ct=2))
            nc.sync.dma_start(b8[:, 0:6], qkvb_d.ap()[:].rearrange(
                "(m p) one -> p (m one)", m=6))
            nc.sync.dma_start(b8[:, 6:8], projb_d.ap()[:].rearrange(
                "(oc p) one -> p (oc one)", oc=2))
            nc.sync.dma_start(projT2p[:], projT_d.ap()[:].rearrange(
                "(p2 u) d o -> (u d) p2 o", p2=2))
            nc.gpsimd.dma_start(gmat_sb[:], gmat_d.ap()[:])
            nc.gpsimd.dma_start(gmatT_sb[:], gmatT_d.ap()[:])
            # bf16 x (residual only, queries half) behind the fp8 x
            for c4 in range(4):
                for ct in range(2):
                    if c4 == 0 and ct == 0:
                        continue
                    sl = slice(c4 * 512, (c4 + 1) * 512)
                    qdma[(2 * c4 + ct) % 3].dma_start(
                        xb_sb[ct][:, sl], xb_d.ap()[ct * 128:(ct + 1) * 128, sl])
            # PE warm-up: junk matmuls on the first landed chunk keep the
            # HAM activity monitor busy so the real pipeline starts at
            # 2.4GHz instead of cold 1.2GHz.
            junk_ps = psum_s.tile([128, 512], F32, tag="ps", name="ps")
            for _ in range(32):
                nc.tensor.matmul(junk_ps[:], xb_sb[0][0:128, 0:128],
                                 xb_sb[0][:, 0:512], start=True, stop=True)
            for w in range(8):
                for ct in range(2):
                    sh = slice(w * 512, (w + 1) * 512)
                    nc.vector.bn_stats(st_sb[ct][:, w, :], x8[:, ct, sh])
            # ---------------- GroupNorm statistics ----------------
            stats2 = []
            for ct in range(2):
                mv = gn.tile([128, 2], F32, tag=f"mv{ct}", name=f"mv{ct}")
                nc.vector.bn_aggr(mv[:], st_sb[ct][:])
                s2 = gn.tile([128, 2], F32, tag=f"s2{ct}", name=f"s2{ct}")
                nc.vector.tensor_copy(s2[:, 0:1], mv[:, 0:1])
                m2 = gn.tile([128, 1], F32, tag=f"m2{ct}", name=f"m2{ct}")
                nc.vector.tensor_mul(m2[:], mv[:, 0:1], mv[:, 0:1])
                nc.vector.tensor_add(s2[:, 1:2], m2[:], mv[:, 1:2])
                stats2.append(s2)

            # group (mean, E[x^2]) -> per-group rstd via exp(-0.5*ln(var+eps))
            gs_ps, bc_sb = [], []
            vg = gn.tile([16, 2], F32, tag="vg", name="vg")
            for ct in range(2):
                g1 = psum_s.tile([16, 2], F32, tag="ps", name="ps")
                nc.tensor.matmul(g1[:], gmat_sb[:], stats2[ct][:],
                                 start=True, stop=True)
                gsb = gn.tile([16, 2], F32, tag=f"gsb{ct}", name=f"gsb{ct}")
                nc.vector.tensor_copy(gsb[:], g1[:])
                gs_ps.append(gsb)
                m2g = gn.tile([16, 1], F32, tag=f"m2g{ct}", name=f"m2g{ct}")
                nc.vector.tensor_mul(m2g[:], gsb[:, 0:1], gsb[:, 0:1])
                nc.vector.tensor_sub(vg[:, ct:ct + 1], gsb[:, 1:2], m2g[:])
            for _ in range(6):
                nc.tensor.matmul(junk_ps[:], xb_sb[0][0:128, 0:128],
                                 xb_sb[0][:, 0:512], start=True, stop=True)
            lgv = gn.tile([16, 2], F32, tag="lgv", name="lgv")
            nc.scalar.activation(lgv[:], vg[:], Ln, bias=eps_t[:])
            rg = gn.tile([16, 2], F32, tag="rg", name="rg")
            nc.scalar.activation(rg[:], lgv[:], Exp, scale=-0.5)
            for ct in range(2):
                bcv = gn.tile([16, 2], F32, tag=f"bcv{ct}", name=f"bcv{ct}")
                nc.vector.tensor_copy(bcv[:, 0:1], gs_ps[ct][:, 0:1])
                nc.vector.tensor_copy(bcv[:, 1:2], rg[:, ct:ct + 1])
                b1 = psum_s.tile([128, 2], F32, tag="ps", name="ps")
                nc.tensor.matmul(b1[:], gmatT_sb[:], bcv[:],
                                 start=True, stop=True)
                bsb = gn.tile([128, 2], F32, tag=f"bc{ct}", name=f"bc{ct}")
                nc.vector.tensor_copy(bsb[:], b1[:])
                bc_sb.append(bsb)

            for _ in range(8):
                nc.tensor.matmul(junk_ps[:], xb_sb[0][0:128, 0:128],
                                 xb_sb[0][:, 0:512], start=True, stop=True)
            # ------- fold rstd into weights (bf16 for the mu path, fp8 for
            # the convs); bias b2 = b - W'mu -------
            qkvS_sb, mu_bf = [], []
            ws8 = consts.tile([128, 2, 3 * C], FP8E4, tag="ws8", name="ws8")
            for ct in range(2):
                ws = consts.tile([128, 3 * C], BF16, tag=f"qkvS{ct}", name=f"qkvS{ct}")
                nc.vector.tensor_scalar_mul(out=ws[:], in0=qkvT_sb[ct][:],
                                            scalar1=bc_sb[ct][:, 1:2])
                qkvS_sb.append(ws)
                nc.vector.tensor_scalar_mul(out=ws8[:, ct, :], in0=qkvT_sb[ct][:],
                                            scalar1=bc_sb[ct][:, 1:2])
                mb = gn.tile([128, 1], BF16, tag=f"mub{ct}", name=f"mub{ct}")
                nc.vector.tensor_copy(mb[:], bc_sb[ct][:, 0:1])
                mu_bf.append(mb)
            b2_sb = [None] * 6
            for m in (2, 3, 0, 1, 4, 5):     # k biases first: K(0,0) gates S
                wm = psum_s.tile([128, 1], F32, tag="ps", name="ps")
                for ct in range(2):
                    nc.tensor.matmul(wm[:], qkvS_sb[ct][:, m * 128:(m + 1) * 128],
                                     mu_bf[ct][:], start=(ct == 0), stop=(ct == 1))
                b2 = gn.tile([128, 1], F32, tag=f"b2_{m}", name=f"b2_{m}")
                nc.vector.tensor_sub(b2[:], b_in[m][:], wm[:])
                b2_sb[m] = b2
            # bf16 copies of the v-slice biases for the proj-bias fold;
            # head pair (2*p2, 2*p2+1) sits at partitions 0-63 / 64-127.
            bv_p2 = []
            for m in (4, 5):
                bb = gn.tile([128, 1], BF16, tag=f"bv{m}", name=f"bv{m}")
                nc.vector.tensor_copy(bb[:], b2_sb[m][:])
                bv_p2.append(bb)

            # ---------------- SBUF destination tiles ----------------
            k_sb = [data.tile([128, T], BF16, tag=f"k{p}", name=f"k{p}")
                    for p in range(2)]
            q_sb = [data.tile([128, TQ], BF16, tag=f"q{p}", name=f"q{p}")
                    for p in range(2)]
            # V in fp8e4, PV-DoubleRow layout: (s2, h, j, c) with c-stride 80
            # (j-step %16==0 per the s3_lw dual-fp8 rule); c=64 is the ones
            # row that accumulates Z.
            vt_sb = data.tile([128, NS2 * HEADS * 2 * 80], FP8E4,
                              tag="vt", name="vt")
            vt5 = vt_sb[:].rearrange("p (s h j c) -> p s h j c",
                                     s=NS2, h=HEADS, j=2, c=80)
            nc.vector.memset(vt5[:, :, :, :, 64:65], 1.0)

            # pb2[oc] = projb[oc] + sum_h projT_h[:,oc]^T @ bv_h
            pb2_sb = []

            def emit_pb2():
                for oc in range(2):
                    pv = psum_s.tile([128, 1], F32, tag="ps", name="ps")
                    for p2 in range(2):
                        nc.tensor.matmul(pv[:],
                                         projT2p[:, p2, oc * 128:(oc + 1) * 128],
                                         bv_p2[p2][:], start=(p2 == 0), stop=(p2 == 1))
                    pb2 = gn.tile([128, 1], F32, tag=f"pb2_{oc}", name=f"pb2_{oc}")
                    nc.vector.tensor_add(pb2[:], pb_sb[oc][:], pv[:])
                    pb2_sb.append(pb2)

            # ------------- conv units (fp8 DoubleRow, woven in) ----------
            def emit_k(p, t8):
                kv = psum_s.tile([128, 512], F32, tag="ps", name="ps")
                nc.tensor.matmul(
                    kv[:], ws8[:, :, C + p * 128:C + (p + 1) * 128],
                    x8[:, :, t8 * 512:(t8 + 1) * 512],
                    start=True, stop=True, perf_mode=DR)
                # bias add on ScalarE: Lrelu(alpha=1) == identity, in-table
                nc.scalar.activation(
                    k_sb[p][:, t8 * 512:(t8 + 1) * 512], kv[:],
                    Ident, bias=b2_sb[2 + p][:])

            def emit_q(p, t4):
                qp = psum_s.tile([128, 512], F32, tag="ps", name="ps")
                nc.tensor.matmul(
                    qp[:], ws8[:, :, p * 128:(p + 1) * 128],
                    x8[:, :, t4 * 512:(t4 + 1) * 512],
                    start=True, stop=True, perf_mode=DR)
                nc.scalar.activation(
                    q_sb[p][:, t4 * 512:(t4 + 1) * 512], qp[:],
                    Ident, bias=b2_sb[p][:])

            def emit_vt(i, eng):
                vp = psum_s.tile([128, C], F32, tag="ps", name="ps")
                nc.tensor.matmul(
                    vp[:], x8[:, :, i * 128:(i + 1) * 128],
                    ws8[:, :, 2 * C:3 * C],
                    start=True, stop=True, perf_mode=DR)
                dst = vt5[:, i // 2, :, i % 2, 0:64]
                if eng == "s":
                    nc.scalar.copy(dst, vp[:].rearrange("p (h c) -> p h c", c=64))
                else:
                    nc.vector.tensor_copy(dst, vp[:].rearrange("p (h c) -> p h c", c=64))

            # per-(block, chunk) pre-emit schedule of conv units
            sched = {}

            def add_sched(b, i, fn):
                sched.setdefault((b, i), []).append(fn)

            add_sched(0, 0, lambda: emit_vt(0, "s"))
            add_sched(0, 0, lambda: emit_vt(1, "v"))
            add_sched(0, 0, lambda: emit_vt(2, "s"))
            for j in range(3, 32):
                b0c = (j - 3) // 2 + 1          # chunks 1..15, two vts per chunk
                add_sched(0, b0c, (lambda jj: lambda: emit_vt(
                    jj, "s" if jj % 2 else "v"))(j))
            for t8 in range(1, 8):
                add_sched(0, t8, (lambda t: lambda: emit_k(0, t))(t8))
            for t4 in range(1, 4):
                add_sched(t4 - 1, 20, (lambda t: lambda: emit_q(0, t))(t4))
            for t8 in range(8):
                add_sched(1 + t8 // 3, 8 + 3 * (t8 % 3), (lambda t: lambda: emit_k(1, t))(t8))
            for t4 in range(4):
                add_sched(2, 17 + 3 * t4, (lambda t: lambda: emit_q(1, t))(t4))
            add_sched(0, 16, emit_pb2)

            # ---------------- attention ----------------
            # 128-step pipeline (8 blocks x 16 key steps of 256). Per step:
            # two S sub-chunks (128 keys each) + their exps (fp8e5 out), with
            # the lagged PV DoubleRow matmuls interleaved between them so
            # every LDWEIGHTS hides behind the previous stream. Block's PV
            # steps 0+1 are emitted together at the step-1 slot so the a_ps
            # handoff never stalls on the previous block's drain.
            PV_LAG = 7
            ah_sb = {}
            post = {}          # g -> list of closures, run before step g

            def at(g, fn):
                post.setdefault(g, []).append(fn)

            def emit_pv_u(pblk, s2, u, p_t):
                p = pblk // NTT
                h = 2 * p + u
                rhs = p_t[:, u * 1024:(u + 1) * 1024].rearrange(
                    "p (j n) -> p j n", j=2)
                nc.tensor.matmul(
                    a_cur[pblk][:, u * 512:(u + 1) * 512],
                    vt5[:, s2, h, :, 0:65], rhs,
                    start=(s2 == 0), stop=(s2 == NS2 - 1), perf_mode=DR)

            def emit_drain(blk, g):
                p, tt = blk // NTT, blk % NTT
                a_ps = a_cur.pop(blk)
                araw = dn.tile([65, 1024], BF16, tag="araw", name="araw")
                last = (blk == 2 * NTT - 1)
                nc.scalar.copy(araw[:, 0:512], a_ps[:, 0:512])
                if last:
                    nc.vector.tensor_copy(araw[:, 512:1024], a_ps[:, 512:1024])
                else:
                    at(g + 1, lambda: nc.vector.tensor_copy(
                        araw[:, 512:1024], a_ps[:, 512:1024]))

                if last:
                    # pipelined per-half 1/Z off the raw psum accumulator:
                    # scalar/gpsimd/vector FIFOs overlap the halves
                    def fin_last():
                        zl = dn.tile([1, 1024], F32, tag="zl", name="zl")
                        zi = dn.tile([1, 1024], F32, tag="zi", name="zi")
                        d_bc = dn.tile([64, 1024], F32, tag="dbc", name="dbc")
                        ah = ahpool.tile([128, 512], BF16, tag=f"ah{p}_{tt}",
                                         name=f"ah{p}_{tt}")
                        for u in range(2):
                            hs = slice(u * 512, (u + 1) * 512)
                            nc.scalar.activation(zl[:, hs], a_ps[64:65, hs], Ln)
                            nc.scalar.activation(zi[:, hs], zl[:, hs], Exp,
                                                 scale=-1.0)
                            nc.gpsimd.partition_broadcast(d_bc[:, hs], zi[:, hs])
                            nc.vector.tensor_mul(ah[u * 64:(u + 1) * 64, :],
                                                 araw[0:64, hs], d_bc[:, hs])
                        ah_sb[(p, tt)] = ah
                    at(g + 1, fin_last)
                    at(g + 2, make_proj(tt))
                    return

                def ln():
                    zl = dn.tile([1, 1024], F32, tag="zl", name="zl")
                    nc.scalar.activation(zl[:], araw[64:65, :], Ln)

                    def inv_bc():
                        zi = dn.tile([1, 1024], F32, tag="zi", name="zi")
                        nc.scalar.activation(zi[:], zl[:], Exp, scale=-1.0)
                        d_bc = dn.tile([64, 1024], F32, tag="dbc", name="dbc")
                        nc.gpsimd.partition_broadcast(d_bc[:], zi[:])

                        def ah_fin():
                            ah = ahpool.tile([128, 512], BF16, tag=f"ah{p}_{tt}",
                                             name=f"ah{p}_{tt}")
                            for u in range(2):
                                nc.gpsimd.tensor_mul(
                                    ah[u * 64:(u + 1) * 64, :],
                                    araw[0:64, u * 512:(u + 1) * 512],
                                    d_bc[:, u * 512:(u + 1) * 512])
                            ah_sb[(p, tt)] = ah
                        at(g + 7, ah_fin)
                    at(g + 4, inv_bc)
                at(g + 2, ln)
                if p == 1:
                    at(g + 9, make_proj(tt))

            def make_proj(tt):
                def proj():
                    for oc in range(2):
                        pr = psum_s.tile([128, 512], F32, tag="ps", name="ps")
                        for p2 in range(2):
                            nc.tensor.matmul(
                                pr[:], projT2p[:, p2, oc * 128:(oc + 1) * 128],
                                ah_sb[(p2, tt)][:],
                                start=(p2 == 0), stop=(p2 == 1))
                        o1 = ao.tile([128, 512], F32, tag="o1", name="o1")
                        nc.vector.tensor_scalar_add(out=o1[:], in0=pr[:],
                                                    scalar1=pb2_sb[oc][:])
                        o2 = ao.tile([128, 512], F32, tag="o2", name="o2")
                        o2eng = nc.vector if tt == NTT - 1 else nc.gpsimd
                        o2eng.tensor_add(o2[:], o1[:],
                                         xb_sb[oc][:, tt * 512:(tt + 1) * 512])
                        nc.sync.dma_start(
                            out_d.ap()[oc * 128:(oc + 1) * 128,
                                       tt * 512:(tt + 1) * 512], o2[:])
                return proj

            emit_k(0, 0)
            emit_q(0, 0)

            NG = 2 * NTT * NS2          # 128 global steps
            pts = {}
            a_cur = {}

            def pv_units_for(g):
                gp = g - PV_LAG
                if gp < 0:
                    return []
                pblk, ps2 = gp // NS2, gp % NS2
                if ps2 == 0:
                    return []
                units = []
                if ps2 == 1:
                    units += [(pblk, 0, 0), (pblk, 0, 1)]
                units += [(pblk, ps2, 0), (pblk, ps2, 1)]
                return units

            def run_pv_units(units):
                for pblk, ps2, u in units:
                    if pblk not in a_cur:
                        a_cur[pblk] = psum_a.tile([65, 1024], F32,
                                                  tag="pa", name="pa")
                    emit_pv_u(pblk, ps2, u, pts[(pblk, ps2)])
                    if u == 1:
                        pts.pop((pblk, ps2))

            for g in range(NG + PV_LAG + 1):
                for fn in post.pop(g, ()):
                    fn()
                units = pv_units_for(g)
                if g < NG:
                    run_pv_units(units)
                    blk, s2 = g // NS2, g % NS2
                    p, tt = blk // NTT, blk % NTT
                    p_t = ppool.tile([128, 2048], FP8E5, tag="pt", name="pt")
                    p_t4 = p_t[:].rearrange("p (u j n) -> p u j n", u=2, j=2)
                    pts[(blk, s2)] = p_t
                    for j in range(2):
                        i = 2 * s2 + j
                        for fn in sched.pop((blk, i), ()):
                            fn()
                        s_ps = psum_s.tile([128, 1024], F32, tag="ps", name="ps")
                        for u in range(2):
                            nc.tensor.matmul(
                                s_ps[:, u * 512:(u + 1) * 512],
                                k_sb[p][u * 64:(u + 1) * 64, i * 128:(i + 1) * 128],
                                q_sb[p][u * 64:(u + 1) * 64, tt * 512:(tt + 1) * 512],
                                start=True, stop=True,
                                tile_position=(u * 64, 0))
                        dst = p_t4[:, :, j, :]
                        src = s_ps[:].rearrange("p (u n) -> p u n", u=2)
                        if j == 0:
                            nc.scalar.activation(dst, src, Exp,
                                                 scale=float(SCALE))
                        else:
                            nc.vector.tensor_scalar(
                                out=dst.bitcast(U8), in0=src,
                                scalar1=A5, scalar2=B5,
                                op0=mybir.AluOpType.mult,
                                op1=mybir.AluOpType.add)
                else:
                    run_pv_units(units)
                gp = g - PV_LAG
                if gp >= 0 and gp % NS2 == NS2 - 1:
                    emit_drain(gp // NS2, g)
            for g in range(NG + PV_LAG + 1, NG + PV_LAG + 12):
                for fn in post.pop(g, ()):
                    fn()
            assert not post and not sched and not pts

    nc.compile()
    nc.m = get_hw_module(nc.m)
    return nc


def _host_prep(inputs):
    x = np.asarray(inputs["x"], np.float32)
    gn_w = np.asarray(inputs["gn_weight"], np.float32)
    gn_b = np.asarray(inputs["gn_bias"], np.float32)
    qkv_w = np.asarray(inputs["qkv_w"], np.float32)
    qkv_b = np.asarray(inputs["qkv_b"], np.float32)
    proj_w = np.asarray(inputs["proj_w"], np.float32)
    proj_b = np.asarray(inputs["proj_b"], np.float32)

    W_ = qkv_w * gn_w[None, :]
    b_ = qkv_w @ gn_b + qkv_b
    qkvT = np.ascontiguousarray(W_.T).astype(BF)
    projT = np.ascontiguousarray(proj_w.T.reshape(HEADS, D, C)).astype(BF)

    gmat = np.zeros((128, 16), np.float32)
    gmatT = np.zeros((16, 128), np.float32)
    for ch in range(128):
        gmat[ch, ch // GS] = 1.0 / GS
        gmatT[ch // GS, ch] = 1.0
    shared = {
        "qkvT": qkvT,
        "qkvb": b_.reshape(3 * C, 1).astype(np.float32),
        "projT": projT,
        "projb": proj_b.reshape(C, 1).astype(np.float32),
        "gmat": gmat,
        "gmatT": gmatT,
    }
    x3 = x.reshape(B, C, T).astype(BF)
    in_maps = []
    for j in range(8):
        b, hf = j // 2, j % 2
        m = dict(shared)
        if hf == 0:
            xbf = np.ascontiguousarray(x3[b])
        else:
            xbf = np.ascontiguousarray(
                np.concatenate([x3[b][:, TQ:], x3[b][:, :TQ]], axis=1))
        m["x8"] = np.ascontiguousarray(
            xbf.reshape(2, 128, T).transpose(1, 0, 2)).astype(
                ml_dtypes.float8_e4m3fn)
        m["xb"] = np.ascontiguousarray(xbf[:, :TQ])
        in_maps.append(m)
    return x, in_maps


def kernel(**inputs) -> np.ndarray:
    if "nc" not in _CACHED:
        _CACHED["nc"] = _build()
    nc = _CACHED["nc"]
    x, in_maps = _host_prep(inputs)
    res = bass_utils.run_bass_kernel_spmd(nc, in_maps, core_ids=list(range(8)))
    out = np.zeros((B, C, T), np.float32)
    for j in range(8):
        b, hf = j // 2, j % 2
        out[b][:, hf * TQ:(hf + 1) * TQ] = np.asarray(
            res.results[j]["out"], np.float32)
    return out.reshape(B, C, Himg, Wimg)
